# revision 31
# baseline (speedup 1.0000x reference)
"""Trainium2 Bass kernel for multi-head causal self-attention.

Problem: X [4, 2048, 1024] fp32, Wq/Wk/Wv/Wo [1024, 1024], H=16 heads, HD=64.
reference: out = softmax_causal((X@Wq) (X@Wk)^T / 8) (X@Wv) merged @ Wo.

Sharding over 8 NeuronCores: core c handles batch b = c // 2 and head group
hg = c % 2 (8 heads each). Each core computes a partial [2048, 1024] output
(its heads' contribution through Wo's row shard); the host sums the two
partials per batch (the tensor-parallel all-reduce, done during unsharding).

v2 design notes (vs the phase-separated baseline):
  * Projections are interleaved with attention at matmul granularity so the
    PE never idles long enough for the HAM clock gate to re-throttle, and
    the ACT engine's exp throughput (the real constraint of the attention
    inner loop) is overlapped with projection matmuls.
  * Scores for both heads of a pair go into one [128, 2, 512] fp32 PSUM
    tile (2 banks) so a single ACTIVATE handles exp for both heads
    (halves ACT instruction overhead).
  * Causal masking: one batched DVE add of a [128, 2, 128] -30000 triangle
    per diagonal k-block; fully-masked leading columns are simply never
    computed (scores, exp, and AV all operate on [rs:512]).
  * Normalization uses reciprocal_approx_fast (~5x faster than the
    microcoded reciprocal) + gpsimd partition_broadcast.
  * PSUM evacuation (AV accumulators -> SBUF) on DVE, not ACT.
  * dc-major first projection so the PE starts as soon as the first X^T
    transpose chunk lands; X^T DMA issues split across the two HWDGE
    queues (sync + act); exp table preloaded via a dummy activation.
"""

import sys

for _p in ("/opt/trn_rl_repo", "/root/.axon_site/_ro/trn_rl_repo"):
    if _p not in sys.path:
        sys.path.insert(0, _p)

import ml_dtypes
import numpy as np

import concourse.bass as bass
import concourse.mybir as mybir
import concourse.tile as tile
from concourse import bacc
from concourse.bass_utils import run_bass_kernel_spmd

F32 = mybir.dt.float32
BF16 = mybir.dt.bfloat16
EXPF = mybir.ActivationFunctionType.Exp

B, S, D, H = 4, 2048, 1024, 16
HD = D // H           # 64
HL = H // 2           # 8 heads per core
DL = HL * HD          # 512 local proj width
NEG = -30000.0        # causal mask additive value (exp underflows to 0)
VW = 65               # AV lhsT width: 64 V cols + ones col (denominator row)


def build_program(s=S, d=D, hl=HL):
    dl = hl * HD
    n_st = s // 128          # s-tiles (128 rows)
    n_dc = d // 128          # d-chunks (projection contraction)
    n_pc = dl // 128         # partition chunks (= head pairs)
    n_q = s // 512           # q-chunks
    n_cc = d // 512          # out column chunks

    nc = bacc.Bacc("TRN2", target_bir_lowering=False, debug=False)

    # X is fed pre-transposed and the weights pre-tiled by the host so every
    # input DMA is plain and contiguous (the XBAR transpose + scatter
    # rearrange DMAs dominated the ramp otherwise).
    XT = nc.dram_tensor("XT", [d, s], BF16, kind="ExternalInput")
    WQ = nc.dram_tensor("WQ", [128, n_pc, n_dc, 128], BF16,
                        kind="ExternalInput")
    WK = nc.dram_tensor("WK", [128, n_pc, n_dc, 128], BF16,
                        kind="ExternalInput")
    WV = nc.dram_tensor("WV", [128, n_dc, dl], BF16, kind="ExternalInput")
    WO = nc.dram_tensor("WO", [128, n_pc, d], BF16, kind="ExternalInput")
    OUT = nc.dram_tensor("OUT", [s, d], F32, kind="ExternalOutput")

    with tile.TileContext(nc) as tc:
        with tc.tile_pool(name="persist", bufs=1) as persist:
            # [128, 2, 128] additive causal mask for two stacked diagonal
            # blocks: 0 where q >= k else -30000.
            cmask = persist.tile([128, 2, 128], F32)
            nc.gpsimd.memset(cmask[:], 0.0)
            nc.gpsimd.affine_select(
                out=cmask[:], in_=cmask[:],
                compare_op=mybir.AluOpType.is_ge, fill=NEG,
                base=0, pattern=[[0, 2], [1, 128]], channel_multiplier=-1,
            )

            xt = [persist.tile([128, s], BF16, name=f"xt{i}") for i in range(n_dc)]
            wq = persist.tile([128, n_pc, n_dc, 128], BF16, name="wq")
            wk = persist.tile([128, n_pc, n_dc, 128], BF16, name="wk")
            wv = persist.tile([128, n_dc, dl], BF16, name="wv")
            wo = persist.tile([128, n_pc, d], BF16, name="wo")
            qt = [persist.tile([128, s], BF16, name=f"qt{i}") for i in range(n_pc)]
            kt = [persist.tile([128, s], BF16, name=f"kt{i}") for i in range(n_pc)]
            vt = [persist.tile([128, hl, VW], BF16, name=f"vt{i}")
                  for i in range(n_st)]
            ot = [persist.tile([128, s], BF16, name=f"ot{i}") for i in range(n_pc)]

            # All input loads ride the scalar HWDGE queue in dependency-
            # priority order; runtime DMAs (dd/sc/OUT) use the sync queue so
            # they never queue behind these. X^T comes in per-q-chunk column
            # slices: phase 0 only needs columns [0:512] (1 MB), so the first
            # attention unit unblocks ~20us earlier than with whole-tile
            # loads.
            nc.scalar.dma_start(wq[:, 0], WQ.ap()[:, 0])
            nc.scalar.dma_start(xt[0][:, 0:512], XT[0:128, 0:512])
            nc.scalar.dma_start(wk[:, 0], WK.ap()[:, 0])
            for dc in range(1, n_dc):
                nc.scalar.dma_start(
                    xt[dc][:, 0:512], XT[dc * 128:(dc + 1) * 128, 0:512])
            for pc in range(1, n_pc):
                nc.scalar.dma_start(wq[:, pc], WQ.ap()[:, pc])
                nc.scalar.dma_start(wk[:, pc], WK.ap()[:, pc])
            nc.scalar.dma_start(wv[:], WV.ap())
            # later q-chunk slices + wo ride the sync queue: it is idle until
            # the first dd/sc/OUT DMAs (~25us in), and keeping them off the
            # scalar queue means the first exps aren't stuck behind 24 DMA
            # issues.
            for q in range(1, n_q):
                qs = slice(q * 512, (q + 1) * 512)
                for dc in range(n_dc):
                    nc.sync.dma_start(
                        xt[dc][:, qs], XT[dc * 128:(dc + 1) * 128, qs])
                if q == 1:
                    nc.sync.dma_start(wo[:], WO.ap())

            # exp table preload: emitting the first (dummy) activation here
            # makes walrus schedule the ~2.7us ACT_TABLE_LOAD during the
            # PE-heavy prologue instead of on the first attention chain.
            scr = persist.tile([128, 8], F32)
            nc.vector.memset(scr[:], 0.0)
            scr2 = persist.tile([128, 8], F32)
            nc.scalar.activation(scr2[:], scr[:], EXPF, scale=1.0)

            with (
                tc.tile_pool(name="pp", bufs=2, space="PSUM") as pp,
                tc.tile_pool(name="sp", bufs=2, space="PSUM") as sp,
                tc.tile_pool(name="avp", bufs=2, space="PSUM") as avp,
                tc.tile_pool(name="work", bufs=3) as work,
                tc.tile_pool(name="norm", bufs=4) as normp,
            ):
                def proj_v(st):
                    ps = pp.tile([128, dl], F32, tag="pp")
                    for dc in range(n_dc):
                        nc.tensor.matmul(
                            ps[:], xt[dc][:, st * 128:(st + 1) * 128],
                            wv[:, dc, :],
                            start=(dc == 0), stop=(dc == n_dc - 1))
                    nc.vector.memset(vt[st][:, :, 64:65], 1.0)
                    nc.vector.tensor_copy(
                        vt[st][:, :, 0:64],
                        ps[:].rearrange("p (h e) -> p h e", h=hl))

                def proj_qk(w, dst, pc, j1):
                    js1 = slice(j1 * 512, (j1 + 1) * 512)
                    ps = pp.tile([128, 512], F32, tag="pp")
                    for dc in range(n_dc):
                        nc.tensor.matmul(
                            ps[:], w[:, pc, dc, :], xt[dc][:, js1],
                            start=(dc == 0), stop=(dc == n_dc - 1))
                    nc.vector.tensor_copy(dst[pc][:, js1], ps[:])

                def out_proj(j, st, cc, pcs, add_to=None, staged=False):
                    """Partial output projection over head pairs `pcs`.
                    Returns the staged SBUF tile (caller DMAs or adds)."""
                    ps = pp.tile([128, 512], F32, tag="pp")
                    for n, pc in enumerate(pcs):
                        nc.tensor.matmul(
                            ps[:], ot[pc][:, st * 128:(st + 1) * 128],
                            wo[:, pc, cc * 512:(cc + 1) * 512],
                            start=(n == 0), stop=(n == len(pcs) - 1))
                    if add_to is None:
                        # the 8 last-chunk partials are all alive at once, so
                        # they get a dedicated 8-deep rotation (a 3-deep one
                        # FIFO-deadlocks DVE behind the final adds).
                        if staged:
                            osb = work.tile([128, 512], F32, tag="osbp",
                                            bufs=8, name=f"osbp{st}_{cc}")
                        else:
                            osb = work.tile([128, 512], F32, tag="osb",
                                            bufs=3, name=f"osb{st}_{cc}")
                        nc.vector.tensor_copy(osb[:], ps[:])
                        return osb
                    nc.vector.tensor_add(add_to[:], add_to[:], ps[:])
                    return add_to

                def dma_out(st, cc, osb):
                    nc.sync.dma_start(
                        OUT[st * 128:(st + 1) * 128, cc * 512:(cc + 1) * 512],
                        osb[:])

                # minimal prologue: just what attn(0, pc0) needs — Q/K for
                # pair 0 and the first four V tiles. The remaining j=0
                # projections ride the phase-0 filler.
                proj_qk(wq, qt, 0, 0)
                proj_qk(wk, kt, 0, 0)
                for st in range(4):
                    proj_v(st)

                for j in range(n_q):
                    js = slice(j * 512, (j + 1) * 512)
                    last_j = j == n_q - 1
                    osb_partial = {}  # (st, cc) -> staged partial for j == last
                    n_i = 4 * j + 4

                    # phase-level filler: always-ready PE work (projections
                    # for the next q-chunk, output projection of the previous
                    # one) drip-fed between attention steps so the PE never
                    # starves while ACT exp gates the dependency chain.
                    filler = []
                    if j == 0:
                        # rest of the j=0 projections, in pc order so each
                        # lands just ahead of its attention unit.
                        for pc in range(1, n_pc):
                            filler.append(
                                lambda pc=pc: proj_qk(wq, qt, pc, 0))
                            filler.append(
                                lambda pc=pc: proj_qk(wk, kt, pc, 0))
                    if j > 0:
                        jp = j - 1
                        for st in range(4 * jp, 4 * jp + 4):
                            for cc in range(n_cc):
                                def og(st=st, cc=cc, jp=jp):
                                    osb = out_proj(
                                        jp, st, cc, list(range(n_pc)))
                                    dma_out(st, cc, osb)
                                filler.append(og)
                    if j + 1 < n_q:
                        for pc in range(n_pc):
                            filler.append(
                                lambda pc=pc, j1=j + 1: proj_qk(wq, qt, pc, j1))
                            filler.append(
                                lambda pc=pc, j1=j + 1: proj_qk(wk, kt, pc, j1))
                        for st in range(4 * (j + 1), 4 * (j + 2)):
                            filler.append(lambda st=st: proj_v(st))
                    n_filler = len(filler) + (8 if last_j else 0)
                    stride = max(1, (n_pc * n_i) // max(1, n_filler))
                    step_ctr = 0

                    for pc in range(n_pc):
                        if last_j and pc == n_pc - 1:
                            # stage the partial output projection over pairs
                            # 0..n-2 while pair n-1 finishes its attention.
                            for st in range(4 * j, 4 * j + 4):
                                for cc in range(n_cc):
                                    def frag(st=st, cc=cc):
                                        osb_partial[(st, cc)] = out_proj(
                                            j, st, cc, list(range(n_pc - 1)),
                                            staged=True)
                                    filler.append(frag)

                        av = [avp.tile([VW, 512], F32, tag="av",
                                       name=f"av{j}_{pc}_{h}") for h in (0, 1)]
                        ets = {}

                        def emit_av(i):
                            r = i - 4 * j
                            rs = max(r, 0) * 128
                            et = ets.pop(i)
                            for h in (0, 1):
                                nc.tensor.matmul(
                                    av[h][:, rs:512], vt[i][:, 2 * pc + h, :],
                                    et[:, h, rs:512],
                                    start=(i == 0), stop=(i == n_i - 1))

                        for i in range(n_i):
                            r = i - 4 * j
                            rs = max(r, 0) * 128
                            stp = sp.tile([128, 2, 512], F32, tag="sp")
                            for h in (0, 1):
                                nc.tensor.matmul(
                                    stp[:, h, rs:512],
                                    kt[pc][64 * h:64 * h + 64,
                                           i * 128:(i + 1) * 128],
                                    qt[pc][64 * h:64 * h + 64,
                                           j * 512 + rs:(j + 1) * 512],
                                    start=True, stop=True,
                                    tile_position=(64 * h, 0))
                            if r >= 0:
                                nc.vector.tensor_add(
                                    stp[:, :, rs:rs + 128],
                                    stp[:, :, rs:rs + 128], cmask[:])
                            et = work.tile([128, 2, 512], BF16, tag="et",
                                           bufs=4)
                            nc.scalar.activation(
                                et[:, :, rs:512], stp[:, :, rs:512], EXPF,
                                scale=0.125)
                            ets[i] = et
                            if i >= 2:
                                emit_av(i - 2)
                            step_ctr += 1
                            if filler and step_ctr % stride == 0:
                                filler.pop(0)()
                        emit_av(n_i - 2)
                        emit_av(n_i - 1)
                        if last_j and pc == n_pc - 1:
                            while filler:
                                filler.pop(0)()

                        # normalization: denominators live in av row 64.
                        # h=1 first so its SBUF->SBUF partition-shift DMA
                        # overlaps h=0's DVE work. Only the very last unit
                        # evacuates the denominator row first (shortens the
                        # kernel tail); elsewhere a single copy releases the
                        # av PSUM bank as fast as possible.
                        tail_unit = last_j and pc == n_pc - 1
                        orw, dd, rr, bc = {}, {}, {}, {}
                        for h in (1, 0):
                            orw[h] = normp.tile([VW, 512], F32, tag="orw",
                                                bufs=4, name=f"orw{j}_{pc}_{h}")
                            dd[h] = normp.tile([1, 512], F32, tag="dd", bufs=4,
                                               name=f"dd{j}_{pc}_{h}")
                            rr[h] = normp.tile([1, 512], F32, tag="rr", bufs=4,
                                               name=f"rr{j}_{pc}_{h}")
                            bc[h] = normp.tile([64, 512], F32, tag="bc", bufs=4,
                                               name=f"bc{j}_{pc}_{h}")
                        if tail_unit:
                            # shortest-latency ordering for the kernel tail:
                            # both denominator rows out first, then the bulk.
                            for h in (1, 0):
                                nc.vector.tensor_copy(
                                    orw[h][64:65, :], av[h][64:65, :])
                                nc.sync.dma_start(dd[h][:], orw[h][64:65, :])
                            for h in (1, 0):
                                nc.vector.reciprocal_approx_fast(
                                    rr[h][:], dd[h][:])
                                nc.gpsimd.partition_broadcast(
                                    bc[h][:], rr[h][:])
                                nc.vector.tensor_copy(
                                    orw[h][0:64, :], av[h][0:64, :])
                        else:
                            for h in (1, 0):
                                nc.vector.tensor_copy(orw[h][:], av[h][:])
                                nc.sync.dma_start(dd[h][:], orw[h][64:65, :])
                                nc.vector.reciprocal_approx_fast(
                                    rr[h][:], dd[h][:])
                                nc.gpsimd.partition_broadcast(
                                    bc[h][:], rr[h][:])
                        for h in (1, 0):
                            if h == 0:
                                nc.vector.tensor_mul(
                                    ot[pc][0:64, js], orw[h][0:64, :],
                                    bc[h][:])
                            else:
                                sc = normp.tile([64, 512], BF16, tag="sc",
                                                bufs=4, name=f"sc{j}_{pc}")
                                nc.vector.tensor_mul(
                                    sc[:], orw[h][0:64, :], bc[h][:])
                                nc.sync.dma_start(ot[pc][64:128, js], sc[:])

                    # drain any leftover filler; the output projection for
                    # this q-chunk rides the NEXT phase's filler (except the
                    # final chunk, completed from the staged partials here).
                    while filler:
                        filler.pop(0)()
                    if last_j:
                        for st in range(4 * j, 4 * j + 4):
                            for cc in range(n_cc):
                                osb = out_proj(j, st, cc, [n_pc - 1],
                                               add_to=osb_partial[(st, cc)])
                                dma_out(st, cc, osb)

    nc.compile()
    return nc


_NC_CACHE = {}


def _get_program():
    key = (S, D, HL)
    if key not in _NC_CACHE:
        _NC_CACHE[key] = build_program()
    return _NC_CACHE[key]


def _bf16(a):
    return np.ascontiguousarray(a.astype(ml_dtypes.bfloat16))


def _wtile(w):
    # [c*128, m] -> [128, c, m]: contraction chunk i lives at [:, i, :]
    c = w.shape[0] // 128
    return np.ascontiguousarray(
        w.reshape(c, 128, w.shape[1]).transpose(1, 0, 2).astype(
            ml_dtypes.bfloat16))


def _wtile_qk(w):
    # [c*128, p*128] -> [128, p, c, 128]: output chunk p is contiguous so the
    # ramp can load just the first head pair's weights.
    c = w.shape[0] // 128
    p = w.shape[1] // 128
    return np.ascontiguousarray(
        w.reshape(c, 128, p, 128).transpose(1, 2, 0, 3).astype(
            ml_dtypes.bfloat16))


def make_in_maps(X, Wq, Wk, Wv, Wo):
    in_maps = []
    for c in range(8):
        b, hg = c // 2, c % 2
        cs = slice(hg * DL, hg * DL + DL)
        in_maps.append({
            "XT": _bf16(X[b].T),
            "WQ": _wtile_qk(Wq[:, cs]),
            "WK": _wtile_qk(Wk[:, cs]),
            "WV": _wtile(Wv[:, cs]),
            "WO": _wtile(Wo[cs, :]),
        })
    return in_maps


def gather_out(results):
    out = np.empty((B, S, D), dtype=np.float32)
    for b in range(B):
        out[b] = results[2 * b]["OUT"] + results[2 * b + 1]["OUT"]
    return out


def kernel(X, Wq, Wk, Wv, Wo):
    X = np.asarray(X, dtype=np.float32)
    Wq = np.asarray(Wq, dtype=np.float32)
    Wk = np.asarray(Wk, dtype=np.float32)
    Wv = np.asarray(Wv, dtype=np.float32)
    Wo = np.asarray(Wo, dtype=np.float32)

    nc = _get_program()
    in_maps = make_in_maps(X, Wq, Wk, Wv, Wo)
    res = run_bass_kernel_spmd(nc, in_maps, list(range(8)), trace=False)
    return gather_out(res.results)


if __name__ == "__main__":
    rng = np.random.default_rng(0)
    scale = 1.0 / np.sqrt(D)
    inputs = {
        "X": rng.standard_normal((B, S, D), dtype=np.float32),
        "Wq": rng.standard_normal((D, D), dtype=np.float32) * scale,
        "Wk": rng.standard_normal((D, D), dtype=np.float32) * scale,
        "Wv": rng.standard_normal((D, D), dtype=np.float32) * scale,
        "Wo": rng.standard_normal((D, D), dtype=np.float32) * scale,
    }
    out = kernel(**inputs)
    print("kernel output shape:", out.shape)


# revision 33
# speedup vs baseline: 1.0372x; 1.0372x over previous
"""Trainium2 Bass kernel for multi-head causal self-attention.

Problem: X [4, 2048, 1024] fp32, Wq/Wk/Wv/Wo [1024, 1024], H=16 heads, HD=64.
reference: out = softmax_causal((X@Wq) (X@Wk)^T / 8) (X@Wv) merged @ Wo.

Sharding over 8 NeuronCores: core c handles batch b = c // 2 and head group
hg = c % 2 (8 heads each). Each core computes a partial [2048, 1024] output
(its heads' contribution through Wo's row shard); the host sums the two
partials per batch (the tensor-parallel all-reduce, done during unsharding).

v2 design notes (vs the phase-separated baseline):
  * Projections are interleaved with attention at matmul granularity so the
    PE never idles long enough for the HAM clock gate to re-throttle, and
    the ACT engine's exp throughput (the real constraint of the attention
    inner loop) is overlapped with projection matmuls.
  * Scores for both heads of a pair go into one [128, 2, 512] fp32 PSUM
    tile (2 banks) so a single ACTIVATE handles exp for both heads
    (halves ACT instruction overhead).
  * Causal masking: one batched DVE add of a [128, 2, 128] -30000 triangle
    per diagonal k-block; fully-masked leading columns are simply never
    computed (scores, exp, and AV all operate on [rs:512]).
  * Normalization uses reciprocal_approx_fast (~5x faster than the
    microcoded reciprocal) + gpsimd partition_broadcast.
  * PSUM evacuation (AV accumulators -> SBUF) on DVE, not ACT.
  * dc-major first projection so the PE starts as soon as the first X^T
    transpose chunk lands; X^T DMA issues split across the two HWDGE
    queues (sync + act); exp table preloaded via a dummy activation.
"""

import sys

for _p in ("/opt/trn_rl_repo", "/root/.axon_site/_ro/trn_rl_repo"):
    if _p not in sys.path:
        sys.path.insert(0, _p)

import ml_dtypes
import numpy as np

import concourse.bass as bass
import concourse.mybir as mybir
import concourse.tile as tile
from concourse import bacc
from concourse.bass_utils import run_bass_kernel_spmd

F32 = mybir.dt.float32
BF16 = mybir.dt.bfloat16
EXPF = mybir.ActivationFunctionType.Exp

B, S, D, H = 4, 2048, 1024, 16
HD = D // H           # 64
HL = H // 2           # 8 heads per core
DL = HL * HD          # 512 local proj width
NEG = -30000.0        # causal mask additive value (exp underflows to 0)
VW = 65               # AV lhsT width: 64 V cols + ones col (denominator row)


def build_program(s=S, d=D, hl=HL):
    dl = hl * HD
    n_st = s // 128          # s-tiles (128 rows)
    n_dc = d // 128          # d-chunks (projection contraction)
    n_pc = dl // 128         # partition chunks (= head pairs)
    n_q = s // 512           # q-chunks
    n_cc = d // 512          # out column chunks

    nc = bacc.Bacc("TRN2", target_bir_lowering=False, debug=False)

    # X is fed pre-transposed and the weights pre-tiled by the host so every
    # input DMA is plain and contiguous (the XBAR transpose + scatter
    # rearrange DMAs dominated the ramp otherwise).
    XT = nc.dram_tensor("XT", [d, s], BF16, kind="ExternalInput")
    WQ = nc.dram_tensor("WQ", [128, n_pc, n_dc, 128], BF16,
                        kind="ExternalInput")
    WK = nc.dram_tensor("WK", [128, n_pc, n_dc, 128], BF16,
                        kind="ExternalInput")
    WV = nc.dram_tensor("WV", [128, n_dc, dl], BF16, kind="ExternalInput")
    WO = nc.dram_tensor("WO", [128, n_pc, d], BF16, kind="ExternalInput")
    OUT = nc.dram_tensor("OUT", [s, d], F32, kind="ExternalOutput")

    with tile.TileContext(nc) as tc:
        with tc.tile_pool(name="persist", bufs=1) as persist:
            # [128, 2, 128] additive causal mask for two stacked diagonal
            # blocks: 0 where q >= k else -30000.
            cmask = persist.tile([128, 2, 128], F32)
            nc.gpsimd.memset(cmask[:], 0.0)
            nc.gpsimd.affine_select(
                out=cmask[:], in_=cmask[:],
                compare_op=mybir.AluOpType.is_ge, fill=NEG,
                base=0, pattern=[[0, 2], [1, 128]], channel_multiplier=-1,
            )

            xt = [persist.tile([128, s], BF16, name=f"xt{i}") for i in range(n_dc)]
            wq = persist.tile([128, n_pc, n_dc, 128], BF16, name="wq")
            wk = persist.tile([128, n_pc, n_dc, 128], BF16, name="wk")
            wv = persist.tile([128, n_dc, dl], BF16, name="wv")
            wo = persist.tile([128, n_pc, d], BF16, name="wo")
            qt = [persist.tile([128, s], BF16, name=f"qt{i}") for i in range(n_pc)]
            kt = [persist.tile([128, s], BF16, name=f"kt{i}") for i in range(n_pc)]
            vt = [persist.tile([128, hl, VW], BF16, name=f"vt{i}")
                  for i in range(n_st)]
            ot = [persist.tile([128, s], BF16, name=f"ot{i}") for i in range(n_pc)]

            # All input loads ride the scalar HWDGE queue in dependency-
            # priority order; runtime DMAs (dd/sc/OUT) use the sync queue so
            # they never queue behind these. X^T comes in per-q-chunk column
            # slices: phase 0 only needs columns [0:512] (1 MB), so the first
            # attention unit unblocks ~20us earlier than with whole-tile
            # loads.
            nc.scalar.dma_start(wq[:, 0], WQ.ap()[:, 0])
            nc.scalar.dma_start(xt[0][:, 0:512], XT[0:128, 0:512])
            nc.scalar.dma_start(wk[:, 0], WK.ap()[:, 0])
            for dc in range(1, n_dc):
                nc.scalar.dma_start(
                    xt[dc][:, 0:512], XT[dc * 128:(dc + 1) * 128, 0:512])
            for pc in range(1, n_pc):
                nc.scalar.dma_start(wq[:, pc], WQ.ap()[:, pc])
                nc.scalar.dma_start(wk[:, pc], WK.ap()[:, pc])
            nc.scalar.dma_start(wv[:], WV.ap())
            for q in range(1, n_q):
                qs = slice(q * 512, (q + 1) * 512)
                for dc in range(n_dc):
                    nc.scalar.dma_start(
                        xt[dc][:, qs], XT[dc * 128:(dc + 1) * 128, qs])
                if q == 1:
                    nc.scalar.dma_start(wo[:], WO.ap())

            # exp table preload: emitting the first (dummy) activation here
            # makes walrus schedule the ~2.7us ACT_TABLE_LOAD during the
            # PE-heavy prologue instead of on the first attention chain.
            scr = persist.tile([128, 8], F32)
            nc.vector.memset(scr[:], 0.0)
            scr2 = persist.tile([128, 8], F32)
            nc.scalar.activation(scr2[:], scr[:], EXPF, scale=1.0)

            with (
                tc.tile_pool(name="pp", bufs=2, space="PSUM") as pp,
                tc.tile_pool(name="sp", bufs=2, space="PSUM") as sp,
                tc.tile_pool(name="avp", bufs=2, space="PSUM") as avp,
                tc.tile_pool(name="work", bufs=3) as work,
                tc.tile_pool(name="norm", bufs=4) as normp,
            ):
                def proj_v(st):
                    ps = pp.tile([128, dl], F32, tag="pp")
                    for dc in range(n_dc):
                        nc.tensor.matmul(
                            ps[:], xt[dc][:, st * 128:(st + 1) * 128],
                            wv[:, dc, :],
                            start=(dc == 0), stop=(dc == n_dc - 1))
                    nc.vector.memset(vt[st][:, :, 64:65], 1.0)
                    nc.vector.tensor_copy(
                        vt[st][:, :, 0:64],
                        ps[:].rearrange("p (h e) -> p h e", h=hl))

                def proj_qk(w, dst, pc, j1):
                    js1 = slice(j1 * 512, (j1 + 1) * 512)
                    ps = pp.tile([128, 512], F32, tag="pp")
                    for dc in range(n_dc):
                        nc.tensor.matmul(
                            ps[:], w[:, pc, dc, :], xt[dc][:, js1],
                            start=(dc == 0), stop=(dc == n_dc - 1))
                    nc.vector.tensor_copy(dst[pc][:, js1], ps[:])

                def out_proj(j, st, cc, pcs, add_to=None, staged=False):
                    """Partial output projection over head pairs `pcs`.
                    Returns the staged SBUF tile (caller DMAs or adds)."""
                    ps = pp.tile([128, 512], F32, tag="pp")
                    for n, pc in enumerate(pcs):
                        nc.tensor.matmul(
                            ps[:], ot[pc][:, st * 128:(st + 1) * 128],
                            wo[:, pc, cc * 512:(cc + 1) * 512],
                            start=(n == 0), stop=(n == len(pcs) - 1))
                    if add_to is None:
                        # the 8 last-chunk partials are all alive at once, so
                        # they get a dedicated 8-deep rotation (a 3-deep one
                        # FIFO-deadlocks DVE behind the final adds).
                        if staged:
                            osb = work.tile([128, 512], F32, tag="osbp",
                                            bufs=8, name=f"osbp{st}_{cc}")
                        else:
                            osb = work.tile([128, 512], F32, tag="osb",
                                            bufs=3, name=f"osb{st}_{cc}")
                        nc.vector.tensor_copy(osb[:], ps[:])
                        return osb
                    nc.vector.tensor_add(add_to[:], add_to[:], ps[:])
                    return add_to

                def dma_out(st, cc, osb):
                    nc.sync.dma_start(
                        OUT[st * 128:(st + 1) * 128, cc * 512:(cc + 1) * 512],
                        osb[:])

                # minimal prologue: just what attn(0, pc0) needs — Q/K for
                # pair 0 and the first four V tiles. The remaining j=0
                # projections ride the phase-0 filler.
                proj_qk(wq, qt, 0, 0)
                proj_qk(wk, kt, 0, 0)
                for st in range(4):
                    proj_v(st)

                for j in range(n_q):
                    js = slice(j * 512, (j + 1) * 512)
                    last_j = j == n_q - 1
                    osb_partial = {}  # (st, cc) -> staged partial for j == last
                    n_i = 4 * j + 4

                    # phase-level filler: always-ready PE work (projections
                    # for the next q-chunk, output projection of the previous
                    # one) drip-fed between attention steps so the PE never
                    # starves while ACT exp gates the dependency chain.
                    filler = []
                    if j == 0:
                        # rest of the j=0 projections, in pc order so each
                        # lands just ahead of its attention unit.
                        for pc in range(1, n_pc):
                            filler.append(
                                lambda pc=pc: proj_qk(wq, qt, pc, 0))
                            filler.append(
                                lambda pc=pc: proj_qk(wk, kt, pc, 0))
                    if j > 0:
                        jp = j - 1
                        for st in range(4 * jp, 4 * jp + 4):
                            for cc in range(n_cc):
                                def og(st=st, cc=cc, jp=jp):
                                    osb = out_proj(
                                        jp, st, cc, list(range(n_pc)))
                                    dma_out(st, cc, osb)
                                filler.append(og)
                    if j + 1 < n_q:
                        for pc in range(n_pc):
                            filler.append(
                                lambda pc=pc, j1=j + 1: proj_qk(wq, qt, pc, j1))
                            filler.append(
                                lambda pc=pc, j1=j + 1: proj_qk(wk, kt, pc, j1))
                        for st in range(4 * (j + 1), 4 * (j + 2)):
                            filler.append(lambda st=st: proj_v(st))
                    n_filler = len(filler) + (8 if last_j else 0)
                    stride = max(1, (n_pc * n_i) // max(1, n_filler))
                    step_ctr = 0

                    for pc in range(n_pc):
                        if last_j and pc == n_pc - 1:
                            # stage the partial output projection over pairs
                            # 0..n-2 while pair n-1 finishes its attention.
                            for st in range(4 * j, 4 * j + 4):
                                for cc in range(n_cc):
                                    def frag(st=st, cc=cc):
                                        osb_partial[(st, cc)] = out_proj(
                                            j, st, cc, list(range(n_pc - 1)),
                                            staged=True)
                                    filler.append(frag)

                        av = [avp.tile([VW, 512], F32, tag="av",
                                       name=f"av{j}_{pc}_{h}") for h in (0, 1)]
                        ets = {}

                        def emit_av(i):
                            r = i - 4 * j
                            rs = max(r, 0) * 128
                            et = ets.pop(i)
                            for h in (0, 1):
                                nc.tensor.matmul(
                                    av[h][:, rs:512], vt[i][:, 2 * pc + h, :],
                                    et[:, h, rs:512],
                                    start=(i == 0), stop=(i == n_i - 1))

                        for i in range(n_i):
                            r = i - 4 * j
                            rs = max(r, 0) * 128
                            stp = sp.tile([128, 2, 512], F32, tag="sp")
                            for h in (0, 1):
                                nc.tensor.matmul(
                                    stp[:, h, rs:512],
                                    kt[pc][64 * h:64 * h + 64,
                                           i * 128:(i + 1) * 128],
                                    qt[pc][64 * h:64 * h + 64,
                                           j * 512 + rs:(j + 1) * 512],
                                    start=True, stop=True,
                                    tile_position=(64 * h, 0))
                            if r >= 0:
                                nc.vector.tensor_add(
                                    stp[:, :, rs:rs + 128],
                                    stp[:, :, rs:rs + 128], cmask[:])
                            et = work.tile([128, 2, 512], BF16, tag="et",
                                           bufs=4)
                            nc.scalar.activation(
                                et[:, :, rs:512], stp[:, :, rs:512], EXPF,
                                scale=0.125)
                            ets[i] = et
                            if i >= 2:
                                emit_av(i - 2)
                            step_ctr += 1
                            if filler and step_ctr % stride == 0:
                                filler.pop(0)()
                        emit_av(n_i - 2)
                        emit_av(n_i - 1)
                        if last_j and pc == n_pc - 1:
                            while filler:
                                filler.pop(0)()

                        # normalization: denominators live in av row 64.
                        # h=1 first so its SBUF->SBUF partition-shift DMA
                        # overlaps h=0's DVE work. Only the very last unit
                        # evacuates the denominator row first (shortens the
                        # kernel tail); elsewhere a single copy releases the
                        # av PSUM bank as fast as possible.
                        tail_unit = last_j and pc == n_pc - 1
                        orw, dd, rr, bc = {}, {}, {}, {}
                        for h in (1, 0):
                            orw[h] = normp.tile([VW, 512], F32, tag="orw",
                                                bufs=4, name=f"orw{j}_{pc}_{h}")
                            dd[h] = normp.tile([1, 512], F32, tag="dd", bufs=4,
                                               name=f"dd{j}_{pc}_{h}")
                            rr[h] = normp.tile([1, 512], F32, tag="rr", bufs=4,
                                               name=f"rr{j}_{pc}_{h}")
                            bc[h] = normp.tile([64, 512], F32, tag="bc", bufs=4,
                                               name=f"bc{j}_{pc}_{h}")
                        if tail_unit:
                            # shortest-latency ordering for the kernel tail:
                            # both denominator rows out first, then the bulk.
                            for h in (1, 0):
                                nc.vector.tensor_copy(
                                    orw[h][64:65, :], av[h][64:65, :])
                                nc.sync.dma_start(dd[h][:], orw[h][64:65, :])
                            for h in (1, 0):
                                nc.vector.reciprocal_approx_fast(
                                    rr[h][:], dd[h][:])
                                nc.gpsimd.partition_broadcast(
                                    bc[h][:], rr[h][:])
                                nc.vector.tensor_copy(
                                    orw[h][0:64, :], av[h][0:64, :])
                            for h in (1, 0):
                                if h == 0:
                                    nc.vector.tensor_mul(
                                        ot[pc][0:64, js], orw[h][0:64, :],
                                        bc[h][:])
                                else:
                                    sc = normp.tile(
                                        [64, 512], BF16, tag="sc",
                                        bufs=4, name=f"sc{j}_{pc}")
                                    nc.vector.tensor_mul(
                                        sc[:], orw[h][0:64, :], bc[h][:])
                                    nc.sync.dma_start(
                                        ot[pc][64:128, js], sc[:])
                        else:
                            for h in (1, 0):
                                nc.vector.tensor_copy(orw[h][:], av[h][:])
                                nc.sync.dma_start(dd[h][:], orw[h][64:65, :])
                                nc.vector.reciprocal_approx_fast(
                                    rr[h][:], dd[h][:])
                                nc.gpsimd.partition_broadcast(
                                    bc[h][:], rr[h][:])
                                if h == 0:
                                    nc.vector.tensor_mul(
                                        ot[pc][0:64, js], orw[h][0:64, :],
                                        bc[h][:])
                                else:
                                    sc = normp.tile(
                                        [64, 512], BF16, tag="sc",
                                        bufs=4, name=f"sc{j}_{pc}")
                                    nc.vector.tensor_mul(
                                        sc[:], orw[h][0:64, :], bc[h][:])
                                    nc.sync.dma_start(
                                        ot[pc][64:128, js], sc[:])

                    # drain any leftover filler; the output projection for
                    # this q-chunk rides the NEXT phase's filler (except the
                    # final chunk, completed from the staged partials here).
                    while filler:
                        filler.pop(0)()
                    if last_j:
                        for st in range(4 * j, 4 * j + 4):
                            for cc in range(n_cc):
                                osb = out_proj(j, st, cc, [n_pc - 1],
                                               add_to=osb_partial[(st, cc)])
                                dma_out(st, cc, osb)

    nc.compile()
    return nc


_NC_CACHE = {}


def _get_program():
    key = (S, D, HL)
    if key not in _NC_CACHE:
        _NC_CACHE[key] = build_program()
    return _NC_CACHE[key]


def _bf16(a):
    return np.ascontiguousarray(a.astype(ml_dtypes.bfloat16))


def _wtile(w):
    # [c*128, m] -> [128, c, m]: contraction chunk i lives at [:, i, :]
    c = w.shape[0] // 128
    return np.ascontiguousarray(
        w.reshape(c, 128, w.shape[1]).transpose(1, 0, 2).astype(
            ml_dtypes.bfloat16))


def _wtile_qk(w):
    # [c*128, p*128] -> [128, p, c, 128]: output chunk p is contiguous so the
    # ramp can load just the first head pair's weights.
    c = w.shape[0] // 128
    p = w.shape[1] // 128
    return np.ascontiguousarray(
        w.reshape(c, 128, p, 128).transpose(1, 2, 0, 3).astype(
            ml_dtypes.bfloat16))


def make_in_maps(X, Wq, Wk, Wv, Wo):
    in_maps = []
    for c in range(8):
        b, hg = c // 2, c % 2
        cs = slice(hg * DL, hg * DL + DL)
        in_maps.append({
            "XT": _bf16(X[b].T),
            "WQ": _wtile_qk(Wq[:, cs]),
            "WK": _wtile_qk(Wk[:, cs]),
            "WV": _wtile(Wv[:, cs]),
            "WO": _wtile(Wo[cs, :]),
        })
    return in_maps


def gather_out(results):
    out = np.empty((B, S, D), dtype=np.float32)
    for b in range(B):
        out[b] = results[2 * b]["OUT"] + results[2 * b + 1]["OUT"]
    return out


def kernel(X, Wq, Wk, Wv, Wo):
    X = np.asarray(X, dtype=np.float32)
    Wq = np.asarray(Wq, dtype=np.float32)
    Wk = np.asarray(Wk, dtype=np.float32)
    Wv = np.asarray(Wv, dtype=np.float32)
    Wo = np.asarray(Wo, dtype=np.float32)

    nc = _get_program()
    in_maps = make_in_maps(X, Wq, Wk, Wv, Wo)
    res = run_bass_kernel_spmd(nc, in_maps, list(range(8)), trace=False)
    return gather_out(res.results)


if __name__ == "__main__":
    rng = np.random.default_rng(0)
    scale = 1.0 / np.sqrt(D)
    inputs = {
        "X": rng.standard_normal((B, S, D), dtype=np.float32),
        "Wq": rng.standard_normal((D, D), dtype=np.float32) * scale,
        "Wk": rng.standard_normal((D, D), dtype=np.float32) * scale,
        "Wv": rng.standard_normal((D, D), dtype=np.float32) * scale,
        "Wo": rng.standard_normal((D, D), dtype=np.float32) * scale,
    }
    out = kernel(**inputs)
    print("kernel output shape:", out.shape)


# revision 34
# speedup vs baseline: 1.0374x; 1.0002x over previous
"""Trainium2 Bass kernel for multi-head causal self-attention.

Problem: X [4, 2048, 1024] fp32, Wq/Wk/Wv/Wo [1024, 1024], H=16 heads, HD=64.
reference: out = softmax_causal((X@Wq) (X@Wk)^T / 8) (X@Wv) merged @ Wo.

Sharding over 8 NeuronCores: core c handles batch b = c // 2 and head group
hg = c % 2 (8 heads each). Each core computes a partial [2048, 1024] output
(its heads' contribution through Wo's row shard); the host sums the two
partials per batch (the tensor-parallel all-reduce, done during unsharding).

v2 design notes (vs the phase-separated baseline):
  * Projections are interleaved with attention at matmul granularity so the
    PE never idles long enough for the HAM clock gate to re-throttle, and
    the ACT engine's exp throughput (the real constraint of the attention
    inner loop) is overlapped with projection matmuls.
  * Scores for both heads of a pair go into one [128, 2, 512] fp32 PSUM
    tile (2 banks) so a single ACTIVATE handles exp for both heads
    (halves ACT instruction overhead).
  * Causal masking: one batched DVE add of a [128, 2, 128] -30000 triangle
    per diagonal k-block; fully-masked leading columns are simply never
    computed (scores, exp, and AV all operate on [rs:512]).
  * Normalization uses reciprocal_approx_fast (~5x faster than the
    microcoded reciprocal) + gpsimd partition_broadcast.
  * PSUM evacuation (AV accumulators -> SBUF) on DVE, not ACT.
  * dc-major first projection so the PE starts as soon as the first X^T
    transpose chunk lands; X^T DMA issues split across the two HWDGE
    queues (sync + act); exp table preloaded via a dummy activation.
"""

import sys

for _p in ("/opt/trn_rl_repo", "/root/.axon_site/_ro/trn_rl_repo"):
    if _p not in sys.path:
        sys.path.insert(0, _p)

import ml_dtypes
import numpy as np

import concourse.bass as bass
import concourse.mybir as mybir
import concourse.tile as tile
from concourse import bacc
from concourse.bass_utils import run_bass_kernel_spmd

F32 = mybir.dt.float32
BF16 = mybir.dt.bfloat16
EXPF = mybir.ActivationFunctionType.Exp

B, S, D, H = 4, 2048, 1024, 16
HD = D // H           # 64
HL = H // 2           # 8 heads per core
DL = HL * HD          # 512 local proj width
NEG = -30000.0        # causal mask additive value (exp underflows to 0)
VW = 65               # AV lhsT width: 64 V cols + ones col (denominator row)


def build_program(s=S, d=D, hl=HL):
    dl = hl * HD
    n_st = s // 128          # s-tiles (128 rows)
    n_dc = d // 128          # d-chunks (projection contraction)
    n_pc = dl // 128         # partition chunks (= head pairs)
    n_q = s // 512           # q-chunks
    n_cc = d // 512          # out column chunks

    nc = bacc.Bacc("TRN2", target_bir_lowering=False, debug=False)

    # X is fed pre-transposed and the weights pre-tiled by the host so every
    # input DMA is plain and contiguous (the XBAR transpose + scatter
    # rearrange DMAs dominated the ramp otherwise).
    XT = nc.dram_tensor("XT", [d, s], BF16, kind="ExternalInput")
    WQ = nc.dram_tensor("WQ", [128, n_pc, n_dc, 128], BF16,
                        kind="ExternalInput")
    WK = nc.dram_tensor("WK", [128, n_pc, n_dc, 128], BF16,
                        kind="ExternalInput")
    WV = nc.dram_tensor("WV", [128, n_dc, dl], BF16, kind="ExternalInput")
    WO = nc.dram_tensor("WO", [128, n_pc, d], BF16, kind="ExternalInput")
    OUT = nc.dram_tensor("OUT", [s, d], F32, kind="ExternalOutput")

    with tile.TileContext(nc) as tc:
        with tc.tile_pool(name="persist", bufs=1) as persist:
            # [128, 2, 128] additive causal mask for two stacked diagonal
            # blocks: 0 where q >= k else -30000.
            cmask = persist.tile([128, 2, 128], F32)
            nc.gpsimd.memset(cmask[:], 0.0)
            nc.gpsimd.affine_select(
                out=cmask[:], in_=cmask[:],
                compare_op=mybir.AluOpType.is_ge, fill=NEG,
                base=0, pattern=[[0, 2], [1, 128]], channel_multiplier=-1,
            )

            xt = [persist.tile([128, s], BF16, name=f"xt{i}") for i in range(n_dc)]
            wq = persist.tile([128, n_pc, n_dc, 128], BF16, name="wq")
            wk = persist.tile([128, n_pc, n_dc, 128], BF16, name="wk")
            wv = persist.tile([128, n_dc, dl], BF16, name="wv")
            wo = persist.tile([128, n_pc, d], BF16, name="wo")
            qt = [persist.tile([128, s], BF16, name=f"qt{i}") for i in range(n_pc)]
            kt = [persist.tile([128, s], BF16, name=f"kt{i}") for i in range(n_pc)]
            vt = [persist.tile([128, hl, VW], BF16, name=f"vt{i}")
                  for i in range(n_st)]
            ot = [persist.tile([128, s], BF16, name=f"ot{i}") for i in range(n_pc)]

            # All input loads ride the scalar HWDGE queue in dependency-
            # priority order; runtime DMAs (dd/sc/OUT) use the sync queue so
            # they never queue behind these. X^T comes in per-q-chunk column
            # slices: phase 0 only needs columns [0:512] (1 MB), so the first
            # attention unit unblocks ~20us earlier than with whole-tile
            # loads.
            nc.scalar.dma_start(wq[:, 0], WQ.ap()[:, 0])
            nc.scalar.dma_start(xt[0][:, 0:512], XT[0:128, 0:512])
            nc.scalar.dma_start(wk[:, 0], WK.ap()[:, 0])
            nc.scalar.dma_start(wv[:], WV.ap())
            for dc in range(1, n_dc):
                nc.scalar.dma_start(
                    xt[dc][:, 0:512], XT[dc * 128:(dc + 1) * 128, 0:512])
            for pc in range(1, n_pc):
                nc.scalar.dma_start(wq[:, pc], WQ.ap()[:, pc])
                nc.scalar.dma_start(wk[:, pc], WK.ap()[:, pc])
            for q in range(1, n_q):
                qs = slice(q * 512, (q + 1) * 512)
                for dc in range(n_dc):
                    nc.scalar.dma_start(
                        xt[dc][:, qs], XT[dc * 128:(dc + 1) * 128, qs])
                if q == 1:
                    nc.scalar.dma_start(wo[:], WO.ap())

            # exp table preload: emitting the first (dummy) activation here
            # makes walrus schedule the ~2.7us ACT_TABLE_LOAD during the
            # PE-heavy prologue instead of on the first attention chain.
            scr = persist.tile([128, 8], F32)
            nc.vector.memset(scr[:], 0.0)
            scr2 = persist.tile([128, 8], F32)
            nc.scalar.activation(scr2[:], scr[:], EXPF, scale=1.0)

            with (
                tc.tile_pool(name="pp", bufs=2, space="PSUM") as pp,
                tc.tile_pool(name="sp", bufs=2, space="PSUM") as sp,
                tc.tile_pool(name="avp", bufs=2, space="PSUM") as avp,
                tc.tile_pool(name="work", bufs=3) as work,
                tc.tile_pool(name="norm", bufs=4) as normp,
            ):
                def proj_v(st):
                    ps = pp.tile([128, dl], F32, tag="pp")
                    for dc in range(n_dc):
                        nc.tensor.matmul(
                            ps[:], xt[dc][:, st * 128:(st + 1) * 128],
                            wv[:, dc, :],
                            start=(dc == 0), stop=(dc == n_dc - 1))
                    nc.vector.memset(vt[st][:, :, 64:65], 1.0)
                    nc.vector.tensor_copy(
                        vt[st][:, :, 0:64],
                        ps[:].rearrange("p (h e) -> p h e", h=hl))

                def proj_qk(w, dst, pc, j1):
                    js1 = slice(j1 * 512, (j1 + 1) * 512)
                    ps = pp.tile([128, 512], F32, tag="pp")
                    for dc in range(n_dc):
                        nc.tensor.matmul(
                            ps[:], w[:, pc, dc, :], xt[dc][:, js1],
                            start=(dc == 0), stop=(dc == n_dc - 1))
                    nc.vector.tensor_copy(dst[pc][:, js1], ps[:])

                def out_proj(j, st, cc, pcs, add_to=None, staged=False):
                    """Partial output projection over head pairs `pcs`.
                    Returns the staged SBUF tile (caller DMAs or adds)."""
                    ps = pp.tile([128, 512], F32, tag="pp")
                    for n, pc in enumerate(pcs):
                        nc.tensor.matmul(
                            ps[:], ot[pc][:, st * 128:(st + 1) * 128],
                            wo[:, pc, cc * 512:(cc + 1) * 512],
                            start=(n == 0), stop=(n == len(pcs) - 1))
                    if add_to is None:
                        # the 8 last-chunk partials are all alive at once, so
                        # they get a dedicated 8-deep rotation (a 3-deep one
                        # FIFO-deadlocks DVE behind the final adds).
                        if staged:
                            osb = work.tile([128, 512], F32, tag="osbp",
                                            bufs=8, name=f"osbp{st}_{cc}")
                        else:
                            osb = work.tile([128, 512], F32, tag="osb",
                                            bufs=3, name=f"osb{st}_{cc}")
                        nc.vector.tensor_copy(osb[:], ps[:])
                        return osb
                    nc.vector.tensor_add(add_to[:], add_to[:], ps[:])
                    return add_to

                def dma_out(st, cc, osb):
                    nc.sync.dma_start(
                        OUT[st * 128:(st + 1) * 128, cc * 512:(cc + 1) * 512],
                        osb[:])

                # minimal prologue: just what attn(0, pc0) needs — Q/K for
                # pair 0 and the first four V tiles. The remaining j=0
                # projections ride the phase-0 filler.
                proj_qk(wq, qt, 0, 0)
                proj_qk(wk, kt, 0, 0)
                for st in range(4):
                    proj_v(st)

                for j in range(n_q):
                    js = slice(j * 512, (j + 1) * 512)
                    last_j = j == n_q - 1
                    osb_partial = {}  # (st, cc) -> staged partial for j == last
                    n_i = 4 * j + 4

                    # phase-level filler: always-ready PE work (projections
                    # for the next q-chunk, output projection of the previous
                    # one) drip-fed between attention steps so the PE never
                    # starves while ACT exp gates the dependency chain.
                    filler = []
                    if j == 0:
                        # rest of the j=0 projections, in pc order so each
                        # lands just ahead of its attention unit.
                        for pc in range(1, n_pc):
                            filler.append(
                                lambda pc=pc: proj_qk(wq, qt, pc, 0))
                            filler.append(
                                lambda pc=pc: proj_qk(wk, kt, pc, 0))
                    if j > 0:
                        jp = j - 1
                        for st in range(4 * jp, 4 * jp + 4):
                            for cc in range(n_cc):
                                def og(st=st, cc=cc, jp=jp):
                                    osb = out_proj(
                                        jp, st, cc, list(range(n_pc)))
                                    dma_out(st, cc, osb)
                                filler.append(og)
                    if j + 1 < n_q:
                        for pc in range(n_pc):
                            filler.append(
                                lambda pc=pc, j1=j + 1: proj_qk(wq, qt, pc, j1))
                            filler.append(
                                lambda pc=pc, j1=j + 1: proj_qk(wk, kt, pc, j1))
                        for st in range(4 * (j + 1), 4 * (j + 2)):
                            filler.append(lambda st=st: proj_v(st))
                    n_filler = len(filler) + (8 if last_j else 0)
                    stride = max(1, (n_pc * n_i) // max(1, n_filler))
                    step_ctr = 0

                    for pc in range(n_pc):
                        if last_j and pc == n_pc - 1:
                            # stage the partial output projection over pairs
                            # 0..n-2 while pair n-1 finishes its attention.
                            for st in range(4 * j, 4 * j + 4):
                                for cc in range(n_cc):
                                    def frag(st=st, cc=cc):
                                        osb_partial[(st, cc)] = out_proj(
                                            j, st, cc, list(range(n_pc - 1)),
                                            staged=True)
                                    filler.append(frag)

                        av = [avp.tile([VW, 512], F32, tag="av",
                                       name=f"av{j}_{pc}_{h}") for h in (0, 1)]
                        ets = {}

                        def emit_av(i):
                            r = i - 4 * j
                            rs = max(r, 0) * 128
                            et = ets.pop(i)
                            for h in (0, 1):
                                nc.tensor.matmul(
                                    av[h][:, rs:512], vt[i][:, 2 * pc + h, :],
                                    et[:, h, rs:512],
                                    start=(i == 0), stop=(i == n_i - 1))

                        for i in range(n_i):
                            r = i - 4 * j
                            rs = max(r, 0) * 128
                            stp = sp.tile([128, 2, 512], F32, tag="sp")
                            for h in (0, 1):
                                nc.tensor.matmul(
                                    stp[:, h, rs:512],
                                    kt[pc][64 * h:64 * h + 64,
                                           i * 128:(i + 1) * 128],
                                    qt[pc][64 * h:64 * h + 64,
                                           j * 512 + rs:(j + 1) * 512],
                                    start=True, stop=True,
                                    tile_position=(64 * h, 0))
                            if r >= 0:
                                nc.vector.tensor_add(
                                    stp[:, :, rs:rs + 128],
                                    stp[:, :, rs:rs + 128], cmask[:])
                            et = work.tile([128, 2, 512], BF16, tag="et",
                                           bufs=4)
                            nc.scalar.activation(
                                et[:, :, rs:512], stp[:, :, rs:512], EXPF,
                                scale=0.125)
                            ets[i] = et
                            if i >= 2:
                                emit_av(i - 2)
                            step_ctr += 1
                            if filler and step_ctr % stride == 0:
                                filler.pop(0)()
                        emit_av(n_i - 2)
                        emit_av(n_i - 1)
                        if last_j and pc == n_pc - 1:
                            while filler:
                                filler.pop(0)()

                        # normalization: denominators live in av row 64.
                        # h=1 first so its SBUF->SBUF partition-shift DMA
                        # overlaps h=0's DVE work. Only the very last unit
                        # evacuates the denominator row first (shortens the
                        # kernel tail); elsewhere a single copy releases the
                        # av PSUM bank as fast as possible.
                        tail_unit = last_j and pc == n_pc - 1
                        orw, dd, rr, bc = {}, {}, {}, {}
                        for h in (1, 0):
                            orw[h] = normp.tile([VW, 512], F32, tag="orw",
                                                bufs=4, name=f"orw{j}_{pc}_{h}")
                            dd[h] = normp.tile([1, 512], F32, tag="dd", bufs=4,
                                               name=f"dd{j}_{pc}_{h}")
                            rr[h] = normp.tile([1, 512], F32, tag="rr", bufs=4,
                                               name=f"rr{j}_{pc}_{h}")
                            bc[h] = normp.tile([64, 512], F32, tag="bc", bufs=4,
                                               name=f"bc{j}_{pc}_{h}")
                        if tail_unit:
                            # shortest-latency ordering for the kernel tail:
                            # both denominator rows out first, then the bulk.
                            for h in (1, 0):
                                nc.vector.tensor_copy(
                                    orw[h][64:65, :], av[h][64:65, :])
                                nc.sync.dma_start(dd[h][:], orw[h][64:65, :])
                            for h in (1, 0):
                                nc.vector.reciprocal_approx_fast(
                                    rr[h][:], dd[h][:])
                                nc.gpsimd.partition_broadcast(
                                    bc[h][:], rr[h][:])
                                nc.vector.tensor_copy(
                                    orw[h][0:64, :], av[h][0:64, :])
                            for h in (1, 0):
                                if h == 0:
                                    nc.vector.tensor_mul(
                                        ot[pc][0:64, js], orw[h][0:64, :],
                                        bc[h][:])
                                else:
                                    sc = normp.tile(
                                        [64, 512], BF16, tag="sc",
                                        bufs=4, name=f"sc{j}_{pc}")
                                    nc.vector.tensor_mul(
                                        sc[:], orw[h][0:64, :], bc[h][:])
                                    nc.sync.dma_start(
                                        ot[pc][64:128, js], sc[:])
                        else:
                            for h in (1, 0):
                                nc.vector.tensor_copy(orw[h][:], av[h][:])
                                nc.sync.dma_start(dd[h][:], orw[h][64:65, :])
                                nc.vector.reciprocal_approx_fast(
                                    rr[h][:], dd[h][:])
                                nc.gpsimd.partition_broadcast(
                                    bc[h][:], rr[h][:])
                                if h == 0:
                                    nc.vector.tensor_mul(
                                        ot[pc][0:64, js], orw[h][0:64, :],
                                        bc[h][:])
                                else:
                                    sc = normp.tile(
                                        [64, 512], BF16, tag="sc",
                                        bufs=4, name=f"sc{j}_{pc}")
                                    nc.vector.tensor_mul(
                                        sc[:], orw[h][0:64, :], bc[h][:])
                                    nc.sync.dma_start(
                                        ot[pc][64:128, js], sc[:])

                    # drain any leftover filler; the output projection for
                    # this q-chunk rides the NEXT phase's filler (except the
                    # final chunk, completed from the staged partials here).
                    while filler:
                        filler.pop(0)()
                    if last_j:
                        for st in range(4 * j, 4 * j + 4):
                            for cc in range(n_cc):
                                osb = out_proj(j, st, cc, [n_pc - 1],
                                               add_to=osb_partial[(st, cc)])
                                dma_out(st, cc, osb)

    nc.compile()
    return nc


_NC_CACHE = {}


def _get_program():
    key = (S, D, HL)
    if key not in _NC_CACHE:
        _NC_CACHE[key] = build_program()
    return _NC_CACHE[key]


def _bf16(a):
    return np.ascontiguousarray(a.astype(ml_dtypes.bfloat16))


def _wtile(w):
    # [c*128, m] -> [128, c, m]: contraction chunk i lives at [:, i, :]
    c = w.shape[0] // 128
    return np.ascontiguousarray(
        w.reshape(c, 128, w.shape[1]).transpose(1, 0, 2).astype(
            ml_dtypes.bfloat16))


def _wtile_qk(w):
    # [c*128, p*128] -> [128, p, c, 128]: output chunk p is contiguous so the
    # ramp can load just the first head pair's weights.
    c = w.shape[0] // 128
    p = w.shape[1] // 128
    return np.ascontiguousarray(
        w.reshape(c, 128, p, 128).transpose(1, 2, 0, 3).astype(
            ml_dtypes.bfloat16))


def make_in_maps(X, Wq, Wk, Wv, Wo):
    in_maps = []
    for c in range(8):
        b, hg = c // 2, c % 2
        cs = slice(hg * DL, hg * DL + DL)
        in_maps.append({
            "XT": _bf16(X[b].T),
            "WQ": _wtile_qk(Wq[:, cs]),
            "WK": _wtile_qk(Wk[:, cs]),
            "WV": _wtile(Wv[:, cs]),
            "WO": _wtile(Wo[cs, :]),
        })
    return in_maps


def gather_out(results):
    out = np.empty((B, S, D), dtype=np.float32)
    for b in range(B):
        out[b] = results[2 * b]["OUT"] + results[2 * b + 1]["OUT"]
    return out


def kernel(X, Wq, Wk, Wv, Wo):
    X = np.asarray(X, dtype=np.float32)
    Wq = np.asarray(Wq, dtype=np.float32)
    Wk = np.asarray(Wk, dtype=np.float32)
    Wv = np.asarray(Wv, dtype=np.float32)
    Wo = np.asarray(Wo, dtype=np.float32)

    nc = _get_program()
    in_maps = make_in_maps(X, Wq, Wk, Wv, Wo)
    res = run_bass_kernel_spmd(nc, in_maps, list(range(8)), trace=False)
    return gather_out(res.results)


if __name__ == "__main__":
    rng = np.random.default_rng(0)
    scale = 1.0 / np.sqrt(D)
    inputs = {
        "X": rng.standard_normal((B, S, D), dtype=np.float32),
        "Wq": rng.standard_normal((D, D), dtype=np.float32) * scale,
        "Wk": rng.standard_normal((D, D), dtype=np.float32) * scale,
        "Wv": rng.standard_normal((D, D), dtype=np.float32) * scale,
        "Wo": rng.standard_normal((D, D), dtype=np.float32) * scale,
    }
    out = kernel(**inputs)
    print("kernel output shape:", out.shape)


# revision 38
# speedup vs baseline: 1.0825x; 1.0435x over previous
"""Trainium2 Bass kernel for multi-head causal self-attention.

Problem: X [4, 2048, 1024] fp32, Wq/Wk/Wv/Wo [1024, 1024], H=16 heads, HD=64.
reference: out = softmax_causal((X@Wq) (X@Wk)^T / 8) (X@Wv) merged @ Wo.

Sharding over 8 NeuronCores: core c handles batch b = c // 2 and head group
hg = c % 2 (8 heads each). Each core computes a partial [2048, 1024] output
(its heads' contribution through Wo's row shard); the host sums the two
partials per batch (the tensor-parallel all-reduce, done during unsharding).

v2 design notes (vs the phase-separated baseline):
  * Projections are interleaved with attention at matmul granularity so the
    PE never idles long enough for the HAM clock gate to re-throttle, and
    the ACT engine's exp throughput (the real constraint of the attention
    inner loop) is overlapped with projection matmuls.
  * Scores for both heads of a pair go into one [128, 2, 512] fp32 PSUM
    tile (2 banks) so a single ACTIVATE handles exp for both heads
    (halves ACT instruction overhead).
  * Causal masking: one batched DVE add of a [128, 2, 128] -30000 triangle
    per diagonal k-block; fully-masked leading columns are simply never
    computed (scores, exp, and AV all operate on [rs:512]).
  * Normalization uses reciprocal_approx_fast (~5x faster than the
    microcoded reciprocal) + gpsimd partition_broadcast.
  * PSUM evacuation (AV accumulators -> SBUF) on DVE, not ACT.
  * dc-major first projection so the PE starts as soon as the first X^T
    transpose chunk lands; X^T DMA issues split across the two HWDGE
    queues (sync + act); exp table preloaded via a dummy activation.
"""

import sys

for _p in ("/opt/trn_rl_repo", "/root/.axon_site/_ro/trn_rl_repo"):
    if _p not in sys.path:
        sys.path.insert(0, _p)

import ml_dtypes
import numpy as np

import concourse.bass as bass
import concourse.mybir as mybir
import concourse.tile as tile
from concourse import bacc
from concourse.bass_utils import run_bass_kernel_spmd

F32 = mybir.dt.float32
BF16 = mybir.dt.bfloat16
EXPF = mybir.ActivationFunctionType.Exp

B, S, D, H = 4, 2048, 1024, 16
HD = D // H           # 64
HL = H // 2           # 8 heads per core
DL = HL * HD          # 512 local proj width
NEG = -30000.0        # causal mask additive value (exp underflows to 0)
VW = 65               # AV lhsT width: 64 V cols + ones col (denominator row)


def build_program(s=S, d=D, hl=HL):
    dl = hl * HD
    n_st = s // 128          # s-tiles (128 rows)
    n_dc = d // 128          # d-chunks (projection contraction)
    n_pc = dl // 128         # partition chunks (= head pairs)
    n_q = s // 512           # q-chunks
    n_cc = d // 512          # out column chunks

    nc = bacc.Bacc("TRN2", target_bir_lowering=False, debug=False)

    # X is fed pre-transposed and the weights pre-tiled by the host so every
    # input DMA is plain and contiguous (the XBAR transpose + scatter
    # rearrange DMAs dominated the ramp otherwise).
    XT = nc.dram_tensor("XT", [d, s], BF16, kind="ExternalInput")
    WQ = nc.dram_tensor("WQ", [128, n_pc, n_dc, 128], BF16,
                        kind="ExternalInput")
    WK = nc.dram_tensor("WK", [128, n_pc, n_dc, 128], BF16,
                        kind="ExternalInput")
    WV = nc.dram_tensor("WV", [128, n_dc, dl], BF16, kind="ExternalInput")
    WO = nc.dram_tensor("WO", [128, n_pc, d], BF16, kind="ExternalInput")
    OUT = nc.dram_tensor("OUT", [s, d], F32, kind="ExternalOutput")

    with tile.TileContext(nc) as tc:
        with tc.tile_pool(name="persist", bufs=1) as persist:
            # [128, 2, 128] additive causal mask for two stacked diagonal
            # blocks: 0 where q >= k else -30000.
            cmask = persist.tile([128, 2, 128], F32)
            nc.gpsimd.memset(cmask[:], 0.0)
            nc.gpsimd.affine_select(
                out=cmask[:], in_=cmask[:],
                compare_op=mybir.AluOpType.is_ge, fill=NEG,
                base=0, pattern=[[0, 2], [1, 128]], channel_multiplier=-1,
            )

            xt = [persist.tile([128, s], BF16, name=f"xt{i}") for i in range(n_dc)]
            wq = persist.tile([128, n_pc, n_dc, 128], BF16, name="wq")
            wk = persist.tile([128, n_pc, n_dc, 128], BF16, name="wk")
            wv = persist.tile([128, n_dc, dl], BF16, name="wv")
            wo = persist.tile([128, n_pc, d], BF16, name="wo")
            qt = [persist.tile([128, s], BF16, name=f"qt{i}") for i in range(n_pc)]
            kt = [persist.tile([128, s], BF16, name=f"kt{i}") for i in range(n_pc)]
            vt = [persist.tile([128, hl, VW], BF16, name=f"vt{i}")
                  for i in range(n_st)]
            ot = [persist.tile([128, s], BF16, name=f"ot{i}") for i in range(n_pc)]

            # All input loads ride the scalar HWDGE queue in dependency-
            # priority order; runtime DMAs (dd/sc/OUT) use the sync queue so
            # they never queue behind these. X^T comes in per-q-chunk column
            # slices: phase 0 only needs columns [0:512] (1 MB), so the first
            # attention unit unblocks ~20us earlier than with whole-tile
            # loads.
            nc.scalar.dma_start(wq[:, 0], WQ.ap()[:, 0])
            nc.scalar.dma_start(xt[0][:, 0:512], XT[0:128, 0:512])
            nc.scalar.dma_start(wk[:, 0], WK.ap()[:, 0])
            nc.scalar.dma_start(wv[:], WV.ap())
            for dc in range(1, n_dc):
                nc.scalar.dma_start(
                    xt[dc][:, 0:512], XT[dc * 128:(dc + 1) * 128, 0:512])
            for pc in range(1, n_pc):
                nc.scalar.dma_start(wq[:, pc], WQ.ap()[:, pc])
                nc.scalar.dma_start(wk[:, pc], WK.ap()[:, pc])
            for q in range(1, n_q):
                qs = slice(q * 512, (q + 1) * 512)
                for dc in range(n_dc):
                    nc.scalar.dma_start(
                        xt[dc][:, qs], XT[dc * 128:(dc + 1) * 128, qs])
                if q == 1:
                    nc.scalar.dma_start(wo[:], WO.ap())

            # exp table preload: emitting the first (dummy) activation here
            # makes walrus schedule the ~2.7us ACT_TABLE_LOAD during the
            # PE-heavy prologue instead of on the first attention chain.
            scr = persist.tile([128, 8], F32)
            nc.vector.memset(scr[:], 0.0)
            scr2 = persist.tile([128, 8], F32)
            nc.scalar.activation(scr2[:], scr[:], EXPF, scale=1.0)

            with (
                tc.tile_pool(name="pp", bufs=2, space="PSUM") as pp,
                tc.tile_pool(name="sp", bufs=2, space="PSUM") as sp,
                tc.tile_pool(name="avp", bufs=2, space="PSUM") as avp,
                tc.tile_pool(name="work", bufs=3) as work,
                tc.tile_pool(name="norm", bufs=4) as normp,
            ):
                def proj_v(st):
                    ps = pp.tile([128, dl], F32, tag="pp")
                    for dc in range(n_dc):
                        nc.tensor.matmul(
                            ps[:], xt[dc][:, st * 128:(st + 1) * 128],
                            wv[:, dc, :],
                            start=(dc == 0), stop=(dc == n_dc - 1))
                    nc.vector.memset(vt[st][:, :, 64:65], 1.0)
                    nc.vector.tensor_copy(
                        vt[st][:, :, 0:64],
                        ps[:].rearrange("p (h e) -> p h e", h=hl))

                def proj_qk(w, dst, pc, j1):
                    js1 = slice(j1 * 512, (j1 + 1) * 512)
                    ps = pp.tile([128, 512], F32, tag="pp")
                    for dc in range(n_dc):
                        nc.tensor.matmul(
                            ps[:], w[:, pc, dc, :], xt[dc][:, js1],
                            start=(dc == 0), stop=(dc == n_dc - 1))
                    nc.vector.tensor_copy(dst[pc][:, js1], ps[:])

                def out_proj(j, st, cc, pcs, add_to=None, staged=False):
                    """Partial output projection over head pairs `pcs`.
                    Returns the staged SBUF tile (caller DMAs or adds)."""
                    ps = pp.tile([128, 512], F32, tag="pp")
                    for n, pc in enumerate(pcs):
                        nc.tensor.matmul(
                            ps[:], ot[pc][:, st * 128:(st + 1) * 128],
                            wo[:, pc, cc * 512:(cc + 1) * 512],
                            start=(n == 0), stop=(n == len(pcs) - 1))
                    if add_to is None:
                        # the 8 last-chunk partials are all alive at once, so
                        # they get a dedicated 8-deep rotation (a 3-deep one
                        # FIFO-deadlocks DVE behind the final adds).
                        if staged:
                            osb = work.tile([128, 512], F32, tag="osbp",
                                            bufs=8, name=f"osbp{st}_{cc}")
                        else:
                            osb = work.tile([128, 512], F32, tag="osb",
                                            bufs=3, name=f"osb{st}_{cc}")
                        nc.vector.tensor_copy(osb[:], ps[:])
                        return osb
                    nc.vector.tensor_add(add_to[:], add_to[:], ps[:])
                    return add_to

                def dma_out(st, cc, osb):
                    nc.sync.dma_start(
                        OUT[st * 128:(st + 1) * 128, cc * 512:(cc + 1) * 512],
                        osb[:])

                # minimal prologue: just what attn(0, pc0) needs — Q/K for
                # pair 0 and the first four V tiles. The remaining j=0
                # projections ride the phase-0 filler.
                proj_qk(wq, qt, 0, 0)
                proj_qk(wk, kt, 0, 0)
                for st in range(4):
                    proj_v(st)

                # Unit sequence: the last two q-chunks' attention units are
                # interleaved so attn(3,*)'s exps (the ACT-bound stretch)
                # start ~25us earlier, overlapping attn(2,*)'s PE work.
                units = ([(0, pc) for pc in range(n_pc)]
                         + [(1, pc) for pc in range(n_pc)]
                         + [(2, 0), (2, 1), (3, 0), (2, 2), (3, 1), (2, 3),
                            (3, 2), (3, 3)])

                # filler: always-ready PE work drip-fed between attention
                # steps so the PE never starves while ACT exp gates the
                # dependency chain. Entries tagged with a unit are forced
                # out before that unit's emission (its scores would
                # FIFO-deadlock behind them otherwise).
                def qk_closure(w, dst, pc, j1):
                    return lambda: proj_qk(w, dst, pc, j1)

                def og_closure(jp, st, cc):
                    def og():
                        osb = out_proj(jp, st, cc, list(range(n_pc)))
                        dma_out(st, cc, osb)
                    return og

                appends = {u: [] for u in units}
                for pc in range(1, n_pc):
                    appends[(0, 0)].append(((0, pc), qk_closure(wq, qt, pc, 0)))
                    appends[(0, 0)].append(((0, pc), qk_closure(wk, kt, pc, 0)))
                for j1, host in ((1, (0, 1)), (2, (1, 0)), (3, (2, 0))):
                    for pc in range(n_pc):
                        appends[host].append(
                            ((j1, pc), qk_closure(wq, qt, pc, j1)))
                        appends[host].append(
                            ((j1, pc), qk_closure(wk, kt, pc, j1)))
                    for st in range(4 * j1, 4 * j1 + 4):
                        appends[host].append(((j1, 0), lambda st=st: proj_v(st)))
                for jp, host in ((0, (1, 0)), (1, (2, 0)), (2, (3, 2))):
                    for st in range(4 * jp, 4 * jp + 4):
                        for cc in range(n_cc):
                            appends[host].append((None, og_closure(jp, st, cc)))

                filler = []
                osb_partial = {}  # (st, cc) -> staged partial for the last j
                total_steps = sum(4 * u[0] + 4 for u in units)
                steps_done = 0
                step_ctr = 0
                for j, pc in units:
                    js = slice(j * 512, (j + 1) * 512)
                    n_i = 4 * j + 4
                    tail_unit = j == n_q - 1 and pc == n_pc - 1
                    filler += appends[(j, pc)]
                    rest = []
                    for tag, fn in filler:
                        if tag == (j, pc):
                            fn()
                        else:
                            rest.append((tag, fn))
                    filler = rest
                    if tail_unit:
                        # stage the partial output projection over pairs
                        # 0..n-2 while pair n-1 finishes its attention.
                        for st in range(4 * j, 4 * j + 4):
                            for cc in range(n_cc):
                                def frag(st=st, cc=cc, j=j):
                                    osb_partial[(st, cc)] = out_proj(
                                        j, st, cc, list(range(n_pc - 1)),
                                        staged=True)
                                filler.append((None, frag))
                    stride = max(1, (total_steps - steps_done)
                                 // max(1, len(filler)))
                    steps_done += n_i

                    if True:
                        av = [avp.tile([VW, 512], F32, tag="av",
                                       name=f"av{j}_{pc}_{h}") for h in (0, 1)]
                        ets = {}

                        def emit_av(i):
                            r = i - 4 * j
                            rs = max(r, 0) * 128
                            et = ets.pop(i)
                            for h in (0, 1):
                                nc.tensor.matmul(
                                    av[h][:, rs:512], vt[i][:, 2 * pc + h, :],
                                    et[:, h, rs:512],
                                    start=(i == 0), stop=(i == n_i - 1))

                        for i in range(n_i):
                            r = i - 4 * j
                            rs = max(r, 0) * 128
                            stp = sp.tile([128, 2, 512], F32, tag="sp")
                            for h in (0, 1):
                                nc.tensor.matmul(
                                    stp[:, h, rs:512],
                                    kt[pc][64 * h:64 * h + 64,
                                           i * 128:(i + 1) * 128],
                                    qt[pc][64 * h:64 * h + 64,
                                           j * 512 + rs:(j + 1) * 512],
                                    start=True, stop=True,
                                    tile_position=(64 * h, 0))
                            if r >= 0:
                                nc.vector.tensor_add(
                                    stp[:, :, rs:rs + 128],
                                    stp[:, :, rs:rs + 128], cmask[:])
                            et = work.tile([128, 2, 512], BF16, tag="et",
                                           bufs=4)
                            nc.scalar.activation(
                                et[:, :, rs:512], stp[:, :, rs:512], EXPF,
                                scale=0.125)
                            ets[i] = et
                            if i >= 2:
                                emit_av(i - 2)
                            step_ctr += 1
                            if filler and step_ctr % stride == 0:
                                filler.pop(0)[1]()
                        emit_av(n_i - 2)
                        emit_av(n_i - 1)
                        if tail_unit:
                            while filler:
                                filler.pop(0)[1]()

                        # normalization: denominators live in av row 64.
                        # h=1 first so its SBUF->SBUF partition-shift DMA
                        # overlaps h=0's DVE work. Only the very last unit
                        # evacuates the denominator row first (shortens the
                        # kernel tail); elsewhere a single copy releases the
                        # av PSUM bank as fast as possible.
                        orw, dd, rr, bc = {}, {}, {}, {}
                        for h in (1, 0):
                            orw[h] = normp.tile([VW, 512], F32, tag="orw",
                                                bufs=4, name=f"orw{j}_{pc}_{h}")
                            dd[h] = normp.tile([1, 512], F32, tag="dd", bufs=4,
                                               name=f"dd{j}_{pc}_{h}")
                            rr[h] = normp.tile([1, 512], F32, tag="rr", bufs=4,
                                               name=f"rr{j}_{pc}_{h}")
                            bc[h] = normp.tile([64, 512], F32, tag="bc", bufs=4,
                                               name=f"bc{j}_{pc}_{h}")
                        if tail_unit:
                            # shortest-latency ordering for the kernel tail:
                            # both denominator rows out first, then the bulk.
                            for h in (1, 0):
                                nc.vector.tensor_copy(
                                    orw[h][64:65, :], av[h][64:65, :])
                                nc.sync.dma_start(dd[h][:], orw[h][64:65, :])
                            for h in (1, 0):
                                nc.vector.reciprocal_approx_fast(
                                    rr[h][:], dd[h][:])
                                nc.gpsimd.partition_broadcast(
                                    bc[h][:], rr[h][:])
                                nc.vector.tensor_copy(
                                    orw[h][0:64, :], av[h][0:64, :])
                            for h in (1, 0):
                                if h == 0:
                                    nc.vector.tensor_mul(
                                        ot[pc][0:64, js], orw[h][0:64, :],
                                        bc[h][:])
                                else:
                                    sc = normp.tile(
                                        [64, 512], BF16, tag="sc",
                                        bufs=4, name=f"sc{j}_{pc}")
                                    nc.vector.tensor_mul(
                                        sc[:], orw[h][0:64, :], bc[h][:])
                                    nc.sync.dma_start(
                                        ot[pc][64:128, js], sc[:])
                        else:
                            for h in (1, 0):
                                nc.vector.tensor_copy(orw[h][:], av[h][:])
                                nc.sync.dma_start(dd[h][:], orw[h][64:65, :])
                                nc.vector.reciprocal_approx_fast(
                                    rr[h][:], dd[h][:])
                                nc.gpsimd.partition_broadcast(
                                    bc[h][:], rr[h][:])
                                if h == 0:
                                    nc.vector.tensor_mul(
                                        ot[pc][0:64, js], orw[h][0:64, :],
                                        bc[h][:])
                                else:
                                    sc = normp.tile(
                                        [64, 512], BF16, tag="sc",
                                        bufs=4, name=f"sc{j}_{pc}")
                                    nc.vector.tensor_mul(
                                        sc[:], orw[h][0:64, :], bc[h][:])
                                    nc.sync.dma_start(
                                        ot[pc][64:128, js], sc[:])

                # complete the final q-chunk's output projection from the
                # staged partials (pair n-1's contribution + add + store).
                while filler:
                    filler.pop(0)[1]()
                jf = n_q - 1
                for st in range(4 * jf, 4 * jf + 4):
                    for cc in range(n_cc):
                        osb = out_proj(jf, st, cc, [n_pc - 1],
                                       add_to=osb_partial[(st, cc)])
                        dma_out(st, cc, osb)

    nc.compile()
    return nc


_NC_CACHE = {}


def _get_program():
    key = (S, D, HL)
    if key not in _NC_CACHE:
        _NC_CACHE[key] = build_program()
    return _NC_CACHE[key]


def _bf16(a):
    return np.ascontiguousarray(a.astype(ml_dtypes.bfloat16))


def _wtile(w):
    # [c*128, m] -> [128, c, m]: contraction chunk i lives at [:, i, :]
    c = w.shape[0] // 128
    return np.ascontiguousarray(
        w.reshape(c, 128, w.shape[1]).transpose(1, 0, 2).astype(
            ml_dtypes.bfloat16))


def _wtile_qk(w):
    # [c*128, p*128] -> [128, p, c, 128]: output chunk p is contiguous so the
    # ramp can load just the first head pair's weights.
    c = w.shape[0] // 128
    p = w.shape[1] // 128
    return np.ascontiguousarray(
        w.reshape(c, 128, p, 128).transpose(1, 2, 0, 3).astype(
            ml_dtypes.bfloat16))


def make_in_maps(X, Wq, Wk, Wv, Wo):
    in_maps = []
    for c in range(8):
        b, hg = c // 2, c % 2
        cs = slice(hg * DL, hg * DL + DL)
        in_maps.append({
            "XT": _bf16(X[b].T),
            "WQ": _wtile_qk(Wq[:, cs]),
            "WK": _wtile_qk(Wk[:, cs]),
            "WV": _wtile(Wv[:, cs]),
            "WO": _wtile(Wo[cs, :]),
        })
    return in_maps


def gather_out(results):
    out = np.empty((B, S, D), dtype=np.float32)
    for b in range(B):
        out[b] = results[2 * b]["OUT"] + results[2 * b + 1]["OUT"]
    return out


def kernel(X, Wq, Wk, Wv, Wo):
    X = np.asarray(X, dtype=np.float32)
    Wq = np.asarray(Wq, dtype=np.float32)
    Wk = np.asarray(Wk, dtype=np.float32)
    Wv = np.asarray(Wv, dtype=np.float32)
    Wo = np.asarray(Wo, dtype=np.float32)

    nc = _get_program()
    in_maps = make_in_maps(X, Wq, Wk, Wv, Wo)
    res = run_bass_kernel_spmd(nc, in_maps, list(range(8)), trace=False)
    return gather_out(res.results)


if __name__ == "__main__":
    rng = np.random.default_rng(0)
    scale = 1.0 / np.sqrt(D)
    inputs = {
        "X": rng.standard_normal((B, S, D), dtype=np.float32),
        "Wq": rng.standard_normal((D, D), dtype=np.float32) * scale,
        "Wk": rng.standard_normal((D, D), dtype=np.float32) * scale,
        "Wv": rng.standard_normal((D, D), dtype=np.float32) * scale,
        "Wo": rng.standard_normal((D, D), dtype=np.float32) * scale,
    }
    out = kernel(**inputs)
    print("kernel output shape:", out.shape)


# revision 42
# speedup vs baseline: 1.0832x; 1.0006x over previous
"""Trainium2 Bass kernel for multi-head causal self-attention.

Problem: X [4, 2048, 1024] fp32, Wq/Wk/Wv/Wo [1024, 1024], H=16 heads, HD=64.
reference: out = softmax_causal((X@Wq) (X@Wk)^T / 8) (X@Wv) merged @ Wo.

Sharding over 8 NeuronCores: core c handles batch b = c // 2 and head group
hg = c % 2 (8 heads each). Each core computes a partial [2048, 1024] output
(its heads' contribution through Wo's row shard); the host sums the two
partials per batch (the tensor-parallel all-reduce, done during unsharding).

v2 design notes (vs the phase-separated baseline):
  * Projections are interleaved with attention at matmul granularity so the
    PE never idles long enough for the HAM clock gate to re-throttle, and
    the ACT engine's exp throughput (the real constraint of the attention
    inner loop) is overlapped with projection matmuls.
  * Scores for both heads of a pair go into one [128, 2, 512] fp32 PSUM
    tile (2 banks) so a single ACTIVATE handles exp for both heads
    (halves ACT instruction overhead).
  * Causal masking: one batched DVE add of a [128, 2, 128] -30000 triangle
    per diagonal k-block; fully-masked leading columns are simply never
    computed (scores, exp, and AV all operate on [rs:512]).
  * Normalization uses reciprocal_approx_fast (~5x faster than the
    microcoded reciprocal) + gpsimd partition_broadcast.
  * PSUM evacuation (AV accumulators -> SBUF) on DVE, not ACT.
  * dc-major first projection so the PE starts as soon as the first X^T
    transpose chunk lands; X^T DMA issues split across the two HWDGE
    queues (sync + act); exp table preloaded via a dummy activation.
"""

import sys

for _p in ("/opt/trn_rl_repo", "/root/.axon_site/_ro/trn_rl_repo"):
    if _p not in sys.path:
        sys.path.insert(0, _p)

import ml_dtypes
import numpy as np

import concourse.bass as bass
import concourse.mybir as mybir
import concourse.tile as tile
from concourse import bacc
from concourse.bass_utils import run_bass_kernel_spmd

F32 = mybir.dt.float32
BF16 = mybir.dt.bfloat16
EXPF = mybir.ActivationFunctionType.Exp

B, S, D, H = 4, 2048, 1024, 16
HD = D // H           # 64
HL = H // 2           # 8 heads per core
DL = HL * HD          # 512 local proj width
NEG = -30000.0        # causal mask additive value (exp underflows to 0)
VW = 65               # AV lhsT width: 64 V cols + ones col (denominator row)


def build_program(s=S, d=D, hl=HL):
    dl = hl * HD
    n_st = s // 128          # s-tiles (128 rows)
    n_dc = d // 128          # d-chunks (projection contraction)
    n_pc = dl // 128         # partition chunks (= head pairs)
    n_q = s // 512           # q-chunks
    n_cc = d // 512          # out column chunks

    nc = bacc.Bacc("TRN2", target_bir_lowering=False, debug=False)

    # X is fed pre-transposed and the weights pre-tiled by the host so every
    # input DMA is plain and contiguous (the XBAR transpose + scatter
    # rearrange DMAs dominated the ramp otherwise).
    XT = nc.dram_tensor("XT", [d, s], BF16, kind="ExternalInput")
    WQ = nc.dram_tensor("WQ", [128, n_pc, n_dc, 128], BF16,
                        kind="ExternalInput")
    WK = nc.dram_tensor("WK", [128, n_pc, n_dc, 128], BF16,
                        kind="ExternalInput")
    WV = nc.dram_tensor("WV", [128, n_dc, dl], BF16, kind="ExternalInput")
    WO = nc.dram_tensor("WO", [128, n_pc, d], BF16, kind="ExternalInput")
    OUT = nc.dram_tensor("OUT", [s, d], BF16, kind="ExternalOutput")

    with tile.TileContext(nc) as tc:
        with tc.tile_pool(name="persist", bufs=1) as persist:
            # [128, 2, 128] additive causal mask for two stacked diagonal
            # blocks: 0 where q >= k else -30000.
            cmask = persist.tile([128, 2, 128], F32)
            nc.gpsimd.memset(cmask[:], 0.0)
            nc.gpsimd.affine_select(
                out=cmask[:], in_=cmask[:],
                compare_op=mybir.AluOpType.is_ge, fill=NEG,
                base=0, pattern=[[0, 2], [1, 128]], channel_multiplier=-1,
            )

            xt = [persist.tile([128, s], BF16, name=f"xt{i}") for i in range(n_dc)]
            wq = persist.tile([128, n_pc, n_dc, 128], BF16, name="wq")
            wk = persist.tile([128, n_pc, n_dc, 128], BF16, name="wk")
            wv = persist.tile([128, n_dc, dl], BF16, name="wv")
            wo = persist.tile([128, n_pc, d], BF16, name="wo")
            qt = [persist.tile([128, s], BF16, name=f"qt{i}") for i in range(n_pc)]
            kt = [persist.tile([128, s], BF16, name=f"kt{i}") for i in range(n_pc)]
            vt = [persist.tile([128, hl, VW], BF16, name=f"vt{i}")
                  for i in range(n_st)]
            ot = [persist.tile([128, s], BF16, name=f"ot{i}") for i in range(n_pc)]

            # All input loads ride the scalar HWDGE queue in dependency-
            # priority order; runtime DMAs (dd/sc/OUT) use the sync queue so
            # they never queue behind these. X^T comes in per-q-chunk column
            # slices: phase 0 only needs columns [0:512] (1 MB), so the first
            # attention unit unblocks ~20us earlier than with whole-tile
            # loads.
            nc.scalar.dma_start(wq[:, 0], WQ.ap()[:, 0])
            nc.scalar.dma_start(xt[0][:, 0:512], XT[0:128, 0:512])
            nc.scalar.dma_start(wk[:, 0], WK.ap()[:, 0])
            nc.scalar.dma_start(wv[:], WV.ap())
            for dc in range(1, n_dc):
                nc.scalar.dma_start(
                    xt[dc][:, 0:512], XT[dc * 128:(dc + 1) * 128, 0:512])
            for pc in range(1, n_pc):
                nc.scalar.dma_start(wq[:, pc], WQ.ap()[:, pc])
                nc.scalar.dma_start(wk[:, pc], WK.ap()[:, pc])
            for q in range(1, n_q):
                qs = slice(q * 512, (q + 1) * 512)
                for dc in range(n_dc):
                    nc.scalar.dma_start(
                        xt[dc][:, qs], XT[dc * 128:(dc + 1) * 128, qs])
                if q == 1:
                    nc.scalar.dma_start(wo[:], WO.ap())

            # exp table preload: emitting the first (dummy) activation here
            # makes walrus schedule the ~2.7us ACT_TABLE_LOAD during the
            # PE-heavy prologue instead of on the first attention chain.
            scr = persist.tile([128, 8], F32)
            nc.vector.memset(scr[:], 0.0)
            scr2 = persist.tile([128, 8], F32)
            nc.scalar.activation(scr2[:], scr[:], EXPF, scale=1.0)

            with (
                tc.tile_pool(name="pp", bufs=2, space="PSUM") as pp,
                tc.tile_pool(name="sp", bufs=2, space="PSUM") as sp,
                tc.tile_pool(name="avp", bufs=2, space="PSUM") as avp,
                tc.tile_pool(name="work", bufs=3) as work,
                tc.tile_pool(name="norm", bufs=4) as normp,
            ):
                def proj_v(st):
                    ps = pp.tile([128, dl], F32, tag="pp")
                    for dc in range(n_dc):
                        nc.tensor.matmul(
                            ps[:], xt[dc][:, st * 128:(st + 1) * 128],
                            wv[:, dc, :],
                            start=(dc == 0), stop=(dc == n_dc - 1))
                    nc.vector.memset(vt[st][:, :, 64:65], 1.0)
                    nc.vector.tensor_copy(
                        vt[st][:, :, 0:64],
                        ps[:].rearrange("p (h e) -> p h e", h=hl))

                def proj_qk(w, dst, pc, j1):
                    js1 = slice(j1 * 512, (j1 + 1) * 512)
                    ps = pp.tile([128, 512], F32, tag="pp")
                    for dc in range(n_dc):
                        nc.tensor.matmul(
                            ps[:], w[:, pc, dc, :], xt[dc][:, js1],
                            start=(dc == 0), stop=(dc == n_dc - 1))
                    nc.vector.tensor_copy(dst[pc][:, js1], ps[:])

                def out_proj(j, st, cc, pcs, add_to=None, staged=False):
                    """Partial output projection over head pairs `pcs`.
                    Returns the staged SBUF tile (caller DMAs or adds)."""
                    ps = pp.tile([128, 512], F32, tag="pp")
                    for n, pc in enumerate(pcs):
                        nc.tensor.matmul(
                            ps[:], ot[pc][:, st * 128:(st + 1) * 128],
                            wo[:, pc, cc * 512:(cc + 1) * 512],
                            start=(n == 0), stop=(n == len(pcs) - 1))
                    if add_to is None:
                        # the 8 last-chunk partials are all alive at once, so
                        # they get a dedicated 8-deep rotation (a 3-deep one
                        # FIFO-deadlocks DVE behind the final adds). Staged
                        # partials stay fp32 (they still get pair n-1 added);
                        # finished tiles go straight to bf16 for the store.
                        if staged:
                            osb = work.tile([128, 512], F32, tag="osbp",
                                            bufs=8, name=f"osbp{st}_{cc}")
                        else:
                            osb = work.tile([128, 512], BF16, tag="osb",
                                            bufs=3, name=f"osb{st}_{cc}")
                        nc.vector.tensor_copy(osb[:], ps[:])
                        return osb
                    fin = work.tile([128, 512], BF16, tag="osbf", bufs=8,
                                    name=f"osbf{st}_{cc}")
                    nc.vector.tensor_add(fin[:], add_to[:], ps[:])
                    return fin

                def dma_out(st, cc, osb, eng=None):
                    (eng or nc.sync).dma_start(
                        OUT[st * 128:(st + 1) * 128, cc * 512:(cc + 1) * 512],
                        osb[:])

                # minimal prologue: just what attn(0, pc0) needs — Q/K for
                # pair 0 and the first four V tiles. The remaining j=0
                # projections ride the phase-0 filler.
                proj_qk(wq, qt, 0, 0)
                proj_qk(wk, kt, 0, 0)
                for st in range(4):
                    proj_v(st)

                # Unit sequence: the last two q-chunks' attention units are
                # interleaved so attn(3,*)'s exps (the ACT-bound stretch)
                # start ~25us earlier, overlapping attn(2,*)'s PE work.
                units = ([(0, pc) for pc in range(n_pc)]
                         + [(1, pc) for pc in range(n_pc)]
                         + [(2, 0), (2, 1), (3, 0), (2, 2), (3, 1), (2, 3),
                            (3, 2), (3, 3)])

                # filler: always-ready PE work drip-fed between attention
                # steps so the PE never starves while ACT exp gates the
                # dependency chain. Entries tagged with a unit are forced
                # out before that unit's emission (its scores would
                # FIFO-deadlock behind them otherwise).
                def qk_closure(w, dst, pc, j1):
                    return lambda: proj_qk(w, dst, pc, j1)

                def og_closure(jp, st, cc):
                    def og():
                        osb = out_proj(jp, st, cc, list(range(n_pc)))
                        dma_out(st, cc, osb)
                    return og

                appends = {u: [] for u in units}
                for pc in range(1, n_pc):
                    appends[(0, 0)].append(((0, pc), qk_closure(wq, qt, pc, 0)))
                    appends[(0, 0)].append(((0, pc), qk_closure(wk, kt, pc, 0)))
                for j1, host in ((1, (0, 1)), (2, (1, 0)), (3, (2, 0))):
                    for pc in range(n_pc):
                        appends[host].append(
                            ((j1, pc), qk_closure(wq, qt, pc, j1)))
                        appends[host].append(
                            ((j1, pc), qk_closure(wk, kt, pc, j1)))
                    for st in range(4 * j1, 4 * j1 + 4):
                        appends[host].append(((j1, 0), lambda st=st: proj_v(st)))
                for jp, host in ((0, (1, 0)), (1, (2, 0)), (2, (3, 2))):
                    for st in range(4 * jp, 4 * jp + 4):
                        for cc in range(n_cc):
                            appends[host].append((None, og_closure(jp, st, cc)))

                filler = []
                osb_partial = {}  # (st, cc) -> staged partial for the last j
                total_steps = sum(4 * u[0] + 4 for u in units)
                steps_done = 0
                step_ctr = 0
                for j, pc in units:
                    js = slice(j * 512, (j + 1) * 512)
                    n_i = 4 * j + 4
                    tail_unit = j == n_q - 1 and pc == n_pc - 1
                    filler += appends[(j, pc)]
                    rest = []
                    for tag, fn in filler:
                        if tag == (j, pc):
                            fn()
                        else:
                            rest.append((tag, fn))
                    filler = rest
                    if tail_unit:
                        # stage the partial output projection over pairs
                        # 0..n-2 while pair n-1 finishes its attention.
                        for st in range(4 * j, 4 * j + 4):
                            for cc in range(n_cc):
                                def frag(st=st, cc=cc, j=j):
                                    osb_partial[(st, cc)] = out_proj(
                                        j, st, cc, list(range(n_pc - 1)),
                                        staged=True)
                                filler.append((None, frag))
                    stride = max(1, (total_steps - steps_done)
                                 // max(1, len(filler)))
                    steps_done += n_i

                    if True:
                        av = [avp.tile([VW, 512], F32, tag="av",
                                       name=f"av{j}_{pc}_{h}") for h in (0, 1)]
                        ets = {}

                        def emit_av(i):
                            r = i - 4 * j
                            rs = max(r, 0) * 128
                            et = ets.pop(i)
                            for h in (0, 1):
                                nc.tensor.matmul(
                                    av[h][:, rs:512], vt[i][:, 2 * pc + h, :],
                                    et[:, h, rs:512],
                                    start=(i == 0), stop=(i == n_i - 1))

                        for i in range(n_i):
                            r = i - 4 * j
                            rs = max(r, 0) * 128
                            stp = sp.tile([128, 2, 512], F32, tag="sp")
                            for h in (0, 1):
                                nc.tensor.matmul(
                                    stp[:, h, rs:512],
                                    kt[pc][64 * h:64 * h + 64,
                                           i * 128:(i + 1) * 128],
                                    qt[pc][64 * h:64 * h + 64,
                                           j * 512 + rs:(j + 1) * 512],
                                    start=True, stop=True,
                                    tile_position=(64 * h, 0))
                            if r >= 0:
                                nc.vector.tensor_add(
                                    stp[:, :, rs:rs + 128],
                                    stp[:, :, rs:rs + 128], cmask[:])
                            et = work.tile([128, 2, 512], BF16, tag="et",
                                           bufs=4)
                            nc.scalar.activation(
                                et[:, :, rs:512], stp[:, :, rs:512], EXPF,
                                scale=0.125)
                            ets[i] = et
                            if i >= 2:
                                emit_av(i - 2)
                            step_ctr += 1
                            if filler and step_ctr % stride == 0:
                                filler.pop(0)[1]()
                        emit_av(n_i - 2)
                        emit_av(n_i - 1)
                        if tail_unit:
                            while filler:
                                filler.pop(0)[1]()

                        # normalization: denominators live in av row 64.
                        # h=1 first so its SBUF->SBUF partition-shift DMA
                        # overlaps h=0's DVE work. Only the very last unit
                        # evacuates the denominator row first (shortens the
                        # kernel tail); elsewhere a single copy releases the
                        # av PSUM bank as fast as possible.
                        orw, dd, rr, bc = {}, {}, {}, {}
                        for h in (1, 0):
                            orw[h] = normp.tile([VW, 512], F32, tag="orw",
                                                bufs=4, name=f"orw{j}_{pc}_{h}")
                            dd[h] = normp.tile([1, 512], F32, tag="dd", bufs=4,
                                               name=f"dd{j}_{pc}_{h}")
                            rr[h] = normp.tile([1, 512], F32, tag="rr", bufs=4,
                                               name=f"rr{j}_{pc}_{h}")
                            bc[h] = normp.tile([64, 512], F32, tag="bc", bufs=4,
                                               name=f"bc{j}_{pc}_{h}")
                        if tail_unit:
                            # shortest-latency ordering for the kernel tail:
                            # both denominator rows out first, then the bulk.
                            for h in (1, 0):
                                nc.vector.tensor_copy(
                                    orw[h][64:65, :], av[h][64:65, :])
                                nc.sync.dma_start(dd[h][:], orw[h][64:65, :])
                            for h in (1, 0):
                                nc.vector.reciprocal_approx_fast(
                                    rr[h][:], dd[h][:])
                                nc.gpsimd.partition_broadcast(
                                    bc[h][:], rr[h][:])
                                nc.vector.tensor_copy(
                                    orw[h][0:64, :], av[h][0:64, :])
                            for h in (1, 0):
                                if h == 0:
                                    nc.vector.tensor_mul(
                                        ot[pc][0:64, js], orw[h][0:64, :],
                                        bc[h][:])
                                else:
                                    sc = normp.tile(
                                        [64, 512], BF16, tag="sc",
                                        bufs=4, name=f"sc{j}_{pc}")
                                    nc.vector.tensor_mul(
                                        sc[:], orw[h][0:64, :], bc[h][:])
                                    nc.sync.dma_start(
                                        ot[pc][64:128, js], sc[:])
                        else:
                            for h in (1, 0):
                                nc.vector.tensor_copy(orw[h][:], av[h][:])
                                nc.sync.dma_start(dd[h][:], orw[h][64:65, :])
                                nc.vector.reciprocal_approx_fast(
                                    rr[h][:], dd[h][:])
                                nc.gpsimd.partition_broadcast(
                                    bc[h][:], rr[h][:])
                                if h == 0:
                                    nc.vector.tensor_mul(
                                        ot[pc][0:64, js], orw[h][0:64, :],
                                        bc[h][:])
                                else:
                                    sc = normp.tile(
                                        [64, 512], BF16, tag="sc",
                                        bufs=4, name=f"sc{j}_{pc}")
                                    nc.vector.tensor_mul(
                                        sc[:], orw[h][0:64, :], bc[h][:])
                                    nc.sync.dma_start(
                                        ot[pc][64:128, js], sc[:])

                # complete the final q-chunk's output projection from the
                # staged partials (pair n-1's contribution + add + store).
                while filler:
                    filler.pop(0)[1]()
                jf = n_q - 1
                for st in range(4 * jf, 4 * jf + 4):
                    for cc in range(n_cc):
                        osb = out_proj(jf, st, cc, [n_pc - 1],
                                       add_to=osb_partial[(st, cc)])
                        # exps are done by now, so the idle scalar HWDGE
                        # queue can drain half the final stores.
                        dma_out(st, cc, osb,
                                nc.sync if (st + cc) % 2 == 0 else nc.scalar)

    nc.compile()
    return nc


_NC_CACHE = {}


def _get_program():
    key = (S, D, HL)
    if key not in _NC_CACHE:
        _NC_CACHE[key] = build_program()
    return _NC_CACHE[key]


def _bf16(a):
    return np.ascontiguousarray(a.astype(ml_dtypes.bfloat16))


def _wtile(w):
    # [c*128, m] -> [128, c, m]: contraction chunk i lives at [:, i, :]
    c = w.shape[0] // 128
    return np.ascontiguousarray(
        w.reshape(c, 128, w.shape[1]).transpose(1, 0, 2).astype(
            ml_dtypes.bfloat16))


def _wtile_qk(w):
    # [c*128, p*128] -> [128, p, c, 128]: output chunk p is contiguous so the
    # ramp can load just the first head pair's weights.
    c = w.shape[0] // 128
    p = w.shape[1] // 128
    return np.ascontiguousarray(
        w.reshape(c, 128, p, 128).transpose(1, 2, 0, 3).astype(
            ml_dtypes.bfloat16))


def make_in_maps(X, Wq, Wk, Wv, Wo):
    in_maps = []
    for c in range(8):
        b, hg = c // 2, c % 2
        cs = slice(hg * DL, hg * DL + DL)
        in_maps.append({
            "XT": _bf16(X[b].T),
            "WQ": _wtile_qk(Wq[:, cs]),
            "WK": _wtile_qk(Wk[:, cs]),
            "WV": _wtile(Wv[:, cs]),
            "WO": _wtile(Wo[cs, :]),
        })
    return in_maps


def gather_out(results):
    out = np.empty((B, S, D), dtype=np.float32)
    for b in range(B):
        out[b] = (results[2 * b]["OUT"].astype(np.float32)
                  + results[2 * b + 1]["OUT"].astype(np.float32))
    return out


def kernel(X, Wq, Wk, Wv, Wo):
    X = np.asarray(X, dtype=np.float32)
    Wq = np.asarray(Wq, dtype=np.float32)
    Wk = np.asarray(Wk, dtype=np.float32)
    Wv = np.asarray(Wv, dtype=np.float32)
    Wo = np.asarray(Wo, dtype=np.float32)

    nc = _get_program()
    in_maps = make_in_maps(X, Wq, Wk, Wv, Wo)
    res = run_bass_kernel_spmd(nc, in_maps, list(range(8)), trace=False)
    return gather_out(res.results)


if __name__ == "__main__":
    rng = np.random.default_rng(0)
    scale = 1.0 / np.sqrt(D)
    inputs = {
        "X": rng.standard_normal((B, S, D), dtype=np.float32),
        "Wq": rng.standard_normal((D, D), dtype=np.float32) * scale,
        "Wk": rng.standard_normal((D, D), dtype=np.float32) * scale,
        "Wv": rng.standard_normal((D, D), dtype=np.float32) * scale,
        "Wo": rng.standard_normal((D, D), dtype=np.float32) * scale,
    }
    out = kernel(**inputs)
    print("kernel output shape:", out.shape)


# revision 45
# speedup vs baseline: 1.0977x; 1.0134x over previous
"""Trainium2 Bass kernel for multi-head causal self-attention.

Problem: X [4, 2048, 1024] fp32, Wq/Wk/Wv/Wo [1024, 1024], H=16 heads, HD=64.
reference: out = softmax_causal((X@Wq) (X@Wk)^T / 8) (X@Wv) merged @ Wo.

Sharding over 8 NeuronCores: core c handles batch b = c // 2 and head group
hg = c % 2 (8 heads each). Each core computes a partial [2048, 1024] output
(its heads' contribution through Wo's row shard); the host sums the two
partials per batch (the tensor-parallel all-reduce, done during unsharding).

v2 design notes (vs the phase-separated baseline):
  * Projections are interleaved with attention at matmul granularity so the
    PE never idles long enough for the HAM clock gate to re-throttle, and
    the ACT engine's exp throughput (the real constraint of the attention
    inner loop) is overlapped with projection matmuls.
  * Scores for both heads of a pair go into one [128, 2, 512] fp32 PSUM
    tile (2 banks) so a single ACTIVATE handles exp for both heads
    (halves ACT instruction overhead).
  * Causal masking: one batched DVE add of a [128, 2, 128] -30000 triangle
    per diagonal k-block; fully-masked leading columns are simply never
    computed (scores, exp, and AV all operate on [rs:512]).
  * Normalization uses reciprocal_approx_fast (~5x faster than the
    microcoded reciprocal) + gpsimd partition_broadcast.
  * PSUM evacuation (AV accumulators -> SBUF) on DVE, not ACT.
  * dc-major first projection so the PE starts as soon as the first X^T
    transpose chunk lands; X^T DMA issues split across the two HWDGE
    queues (sync + act); exp table preloaded via a dummy activation.
"""

import sys

for _p in ("/opt/trn_rl_repo", "/root/.axon_site/_ro/trn_rl_repo"):
    if _p not in sys.path:
        sys.path.insert(0, _p)

import ml_dtypes
import numpy as np

import concourse.bass as bass
import concourse.mybir as mybir
import concourse.tile as tile
from concourse import bacc
from concourse.bass_utils import run_bass_kernel_spmd

F32 = mybir.dt.float32
BF16 = mybir.dt.bfloat16
EXPF = mybir.ActivationFunctionType.Exp

B, S, D, H = 4, 2048, 1024, 16
HD = D // H           # 64
HL = H // 2           # 8 heads per core
DL = HL * HD          # 512 local proj width
NEG = -30000.0        # causal mask additive value (exp underflows to 0)
VW = 65               # AV lhsT width: 64 V cols + ones col (denominator row)


def build_program(s=S, d=D, hl=HL):
    dl = hl * HD
    n_st = s // 128          # s-tiles (128 rows)
    n_dc = d // 128          # d-chunks (projection contraction)
    n_pc = dl // 128         # partition chunks (= head pairs)
    n_q = s // 512           # q-chunks
    n_cc = d // 512          # out column chunks

    nc = bacc.Bacc("TRN2", target_bir_lowering=False, debug=False)

    # X is fed pre-transposed and the weights pre-tiled by the host so every
    # input DMA is plain and contiguous (the XBAR transpose + scatter
    # rearrange DMAs dominated the ramp otherwise).
    XT = nc.dram_tensor("XT", [d, s], BF16, kind="ExternalInput")
    WQ = nc.dram_tensor("WQ", [128, n_pc, n_dc, 128], BF16,
                        kind="ExternalInput")
    WK = nc.dram_tensor("WK", [128, n_pc, n_dc, 128], BF16,
                        kind="ExternalInput")
    WV = nc.dram_tensor("WV", [128, n_dc, dl], BF16, kind="ExternalInput")
    WO = nc.dram_tensor("WO", [128, n_pc, d], BF16, kind="ExternalInput")
    OUT = nc.dram_tensor("OUT", [s, d], F32, kind="ExternalOutput")

    with tile.TileContext(nc) as tc:
        with tc.tile_pool(name="persist", bufs=1) as persist:
            # [128, 2, 128] additive causal mask for two stacked diagonal
            # blocks: 0 where q >= k else -30000.
            cmask = persist.tile([128, 2, 128], F32)
            nc.gpsimd.memset(cmask[:], 0.0)
            nc.gpsimd.affine_select(
                out=cmask[:], in_=cmask[:],
                compare_op=mybir.AluOpType.is_ge, fill=NEG,
                base=0, pattern=[[0, 2], [1, 128]], channel_multiplier=-1,
            )

            xt = [persist.tile([128, s], BF16, name=f"xt{i}") for i in range(n_dc)]
            wq = persist.tile([128, n_pc, n_dc, 128], BF16, name="wq")
            wk = persist.tile([128, n_pc, n_dc, 128], BF16, name="wk")
            wv = persist.tile([128, n_dc, dl], BF16, name="wv")
            wo = persist.tile([128, n_pc, d], BF16, name="wo")
            qt = [persist.tile([128, s], BF16, name=f"qt{i}") for i in range(n_pc)]
            kt = [persist.tile([128, s], BF16, name=f"kt{i}") for i in range(n_pc)]
            vt = [persist.tile([128, hl, VW], BF16, name=f"vt{i}")
                  for i in range(n_st)]
            ot = [persist.tile([128, s], BF16, name=f"ot{i}") for i in range(n_pc)]

            # All input loads ride the scalar HWDGE queue in dependency-
            # priority order; runtime DMAs (dd/sc/OUT) use the sync queue so
            # they never queue behind these. X^T comes in per-q-chunk column
            # slices: phase 0 only needs columns [0:512] (1 MB), so the first
            # attention unit unblocks ~20us earlier than with whole-tile
            # loads.
            nc.scalar.dma_start(wq[:, 0], WQ.ap()[:, 0])
            nc.scalar.dma_start(xt[0][:, 0:512], XT[0:128, 0:512])
            nc.scalar.dma_start(wk[:, 0], WK.ap()[:, 0])
            for dc in range(1, n_dc):
                nc.scalar.dma_start(
                    xt[dc][:, 0:512], XT[dc * 128:(dc + 1) * 128, 0:512])
            nc.scalar.dma_start(wv[:], WV.ap())
            for pc in range(1, n_pc):
                nc.scalar.dma_start(wq[:, pc], WQ.ap()[:, pc])
                nc.scalar.dma_start(wk[:, pc], WK.ap()[:, pc])
            for q in range(1, n_q):
                qs = slice(q * 512, (q + 1) * 512)
                for dc in range(n_dc):
                    nc.scalar.dma_start(
                        xt[dc][:, qs], XT[dc * 128:(dc + 1) * 128, qs])
                if q == 1:
                    nc.scalar.dma_start(wo[:], WO.ap())

            # exp table preload: emitting the first (dummy) activation here
            # makes walrus schedule the ~2.7us ACT_TABLE_LOAD during the
            # PE-heavy prologue instead of on the first attention chain.
            scr = persist.tile([128, 8], F32)
            nc.vector.memset(scr[:], 0.0)
            scr2 = persist.tile([128, 8], F32)
            nc.scalar.activation(scr2[:], scr[:], EXPF, scale=1.0)

            with (
                tc.tile_pool(name="pp", bufs=2, space="PSUM") as pp,
                tc.tile_pool(name="sp", bufs=2, space="PSUM") as sp,
                tc.tile_pool(name="avp", bufs=2, space="PSUM") as avp,
                tc.tile_pool(name="work", bufs=3) as work,
                tc.tile_pool(name="norm", bufs=4) as normp,
            ):
                def proj_v(st):
                    ps = pp.tile([128, dl], F32, tag="pp")
                    for dc in range(n_dc):
                        nc.tensor.matmul(
                            ps[:], xt[dc][:, st * 128:(st + 1) * 128],
                            wv[:, dc, :],
                            start=(dc == 0), stop=(dc == n_dc - 1))
                    nc.vector.memset(vt[st][:, :, 64:65], 1.0)
                    nc.vector.tensor_copy(
                        vt[st][:, :, 0:64],
                        ps[:].rearrange("p (h e) -> p h e", h=hl))

                def proj_qk(w, dst, pc, j1):
                    js1 = slice(j1 * 512, (j1 + 1) * 512)
                    ps = pp.tile([128, 512], F32, tag="pp")
                    for dc in range(n_dc):
                        nc.tensor.matmul(
                            ps[:], w[:, pc, dc, :], xt[dc][:, js1],
                            start=(dc == 0), stop=(dc == n_dc - 1))
                    nc.vector.tensor_copy(dst[pc][:, js1], ps[:])

                def out_proj(j, st, cc, pcs, add_to=None, staged=False):
                    """Partial output projection over head pairs `pcs`.
                    Returns the staged SBUF tile (caller DMAs or adds)."""
                    ps = pp.tile([128, 512], F32, tag="pp")
                    for n, pc in enumerate(pcs):
                        nc.tensor.matmul(
                            ps[:], ot[pc][:, st * 128:(st + 1) * 128],
                            wo[:, pc, cc * 512:(cc + 1) * 512],
                            start=(n == 0), stop=(n == len(pcs) - 1))
                    if add_to is None:
                        # the 8 last-chunk partials are all alive at once, so
                        # they get a dedicated 8-deep rotation (a 3-deep one
                        # FIFO-deadlocks DVE behind the final adds).
                        if staged:
                            osb = work.tile([128, 512], F32, tag="osbp",
                                            bufs=8, name=f"osbp{st}_{cc}")
                        else:
                            osb = work.tile([128, 512], F32, tag="osb",
                                            bufs=3, name=f"osb{st}_{cc}")
                        nc.vector.tensor_copy(osb[:], ps[:])
                        return osb
                    nc.vector.tensor_add(add_to[:], add_to[:], ps[:])
                    return add_to

                def dma_out(st, cc, osb):
                    nc.sync.dma_start(
                        OUT[st * 128:(st + 1) * 128, cc * 512:(cc + 1) * 512],
                        osb[:])

                # minimal prologue: just what attn(0, pc0) needs — Q/K for
                # pair 0 and the first four V tiles. The remaining j=0
                # projections ride the phase-0 filler.
                proj_qk(wq, qt, 0, 0)
                proj_qk(wk, kt, 0, 0)
                for st in range(4):
                    proj_v(st)

                # Unit sequence: the last two q-chunks' attention units are
                # interleaved so attn(3,*)'s exps (the ACT-bound stretch)
                # start ~25us earlier, overlapping attn(2,*)'s PE work.
                units = ([(0, pc) for pc in range(n_pc)]
                         + [(1, pc) for pc in range(n_pc)]
                         + [(2, 0), (2, 1), (3, 0), (2, 2), (3, 1), (2, 3),
                            (3, 2), (3, 3)])

                # filler: always-ready PE work drip-fed between attention
                # steps so the PE never starves while ACT exp gates the
                # dependency chain. Entries tagged with a unit are forced
                # out before that unit's emission (its scores would
                # FIFO-deadlock behind them otherwise).
                def qk_closure(w, dst, pc, j1):
                    return lambda: proj_qk(w, dst, pc, j1)

                def og_closure(jp, st, cc):
                    def og():
                        osb = out_proj(jp, st, cc, list(range(n_pc)))
                        dma_out(st, cc, osb)
                    return og

                appends = {u: [] for u in units}
                for pc in range(1, n_pc):
                    appends[(0, 0)].append(((0, pc), qk_closure(wq, qt, pc, 0)))
                    appends[(0, 0)].append(((0, pc), qk_closure(wk, kt, pc, 0)))
                for j1, host in ((1, (0, 1)), (2, (1, 0)), (3, (2, 0))):
                    for pc in range(n_pc):
                        appends[host].append(
                            ((j1, pc), qk_closure(wq, qt, pc, j1)))
                        appends[host].append(
                            ((j1, pc), qk_closure(wk, kt, pc, j1)))
                    for st in range(4 * j1, 4 * j1 + 4):
                        appends[host].append(((j1, 0), lambda st=st: proj_v(st)))
                for jp, host in ((0, (1, 0)), (1, (2, 0)), (2, (3, 2))):
                    for st in range(4 * jp, 4 * jp + 4):
                        for cc in range(n_cc):
                            appends[host].append((None, og_closure(jp, st, cc)))

                filler = []
                osb_partial = {}  # (st, cc) -> staged partial for the last j
                total_steps = sum(4 * u[0] + 4 for u in units)
                steps_done = 0
                step_ctr = 0
                for j, pc in units:
                    js = slice(j * 512, (j + 1) * 512)
                    n_i = 4 * j + 4
                    tail_unit = j == n_q - 1 and pc == n_pc - 1
                    filler += appends[(j, pc)]
                    rest = []
                    for tag, fn in filler:
                        if tag == (j, pc):
                            fn()
                        else:
                            rest.append((tag, fn))
                    filler = rest
                    if tail_unit:
                        # stage the partial output projection over pairs
                        # 0..n-2 while pair n-1 finishes its attention.
                        for st in range(4 * j, 4 * j + 4):
                            for cc in range(n_cc):
                                def frag(st=st, cc=cc, j=j):
                                    osb_partial[(st, cc)] = out_proj(
                                        j, st, cc, list(range(n_pc - 1)),
                                        staged=True)
                                filler.append((None, frag))
                    stride = max(1, (total_steps - steps_done)
                                 // max(1, len(filler)))
                    steps_done += n_i

                    if True:
                        av = [avp.tile([VW, 512], F32, tag="av",
                                       name=f"av{j}_{pc}_{h}") for h in (0, 1)]
                        ets = {}

                        def emit_av(i):
                            r = i - 4 * j
                            rs = max(r, 0) * 128
                            et = ets.pop(i)
                            for h in (0, 1):
                                nc.tensor.matmul(
                                    av[h][:, rs:512], vt[i][:, 2 * pc + h, :],
                                    et[:, h, rs:512],
                                    start=(i == 0), stop=(i == n_i - 1))

                        for i in range(n_i):
                            r = i - 4 * j
                            rs = max(r, 0) * 128
                            stp = sp.tile([128, 2, 512], F32, tag="sp")
                            for h in (0, 1):
                                nc.tensor.matmul(
                                    stp[:, h, rs:512],
                                    kt[pc][64 * h:64 * h + 64,
                                           i * 128:(i + 1) * 128],
                                    qt[pc][64 * h:64 * h + 64,
                                           j * 512 + rs:(j + 1) * 512],
                                    start=True, stop=True,
                                    tile_position=(64 * h, 0))
                            if r >= 0:
                                nc.vector.tensor_add(
                                    stp[:, :, rs:rs + 128],
                                    stp[:, :, rs:rs + 128], cmask[:])
                            et = work.tile([128, 2, 512], BF16, tag="et",
                                           bufs=4)
                            nc.scalar.activation(
                                et[:, :, rs:512], stp[:, :, rs:512], EXPF,
                                scale=0.125)
                            ets[i] = et
                            if i >= 2:
                                emit_av(i - 2)
                            step_ctr += 1
                            if filler and step_ctr % stride == 0:
                                filler.pop(0)[1]()
                        emit_av(n_i - 2)
                        emit_av(n_i - 1)
                        if tail_unit:
                            while filler:
                                filler.pop(0)[1]()

                        # normalization: denominators live in av row 64.
                        # h=1 first so its SBUF->SBUF partition-shift DMA
                        # overlaps h=0's DVE work. Only the very last unit
                        # evacuates the denominator row first (shortens the
                        # kernel tail); elsewhere a single copy releases the
                        # av PSUM bank as fast as possible.
                        orw, dd, rr, bc = {}, {}, {}, {}
                        for h in (1, 0):
                            orw[h] = normp.tile([VW, 512], F32, tag="orw",
                                                bufs=4, name=f"orw{j}_{pc}_{h}")
                            dd[h] = normp.tile([1, 512], F32, tag="dd", bufs=4,
                                               name=f"dd{j}_{pc}_{h}")
                            rr[h] = normp.tile([1, 512], F32, tag="rr", bufs=4,
                                               name=f"rr{j}_{pc}_{h}")
                            bc[h] = normp.tile([64, 512], F32, tag="bc", bufs=4,
                                               name=f"bc{j}_{pc}_{h}")
                        if tail_unit:
                            # shortest-latency tail: denominator rows out
                            # first, normalize straight from the AV PSUM (no
                            # one reuses those banks after the last unit),
                            # and chunk by 128 columns so the final output
                            # projection starts per-st instead of waiting
                            # for the full 512-wide partition-shift DMA.
                            for h in (1, 0):
                                nc.vector.tensor_copy(
                                    orw[h][64:65, :], av[h][64:65, :])
                                nc.sync.dma_start(dd[h][:], orw[h][64:65, :])
                            for h in (1, 0):
                                nc.vector.reciprocal_approx_fast(
                                    rr[h][:], dd[h][:])
                                nc.gpsimd.partition_broadcast(
                                    bc[h][:], rr[h][:])
                            for ch in range(4):
                                cs = slice(ch * 128, (ch + 1) * 128)
                                ocs = slice(j * 512 + ch * 128,
                                            j * 512 + (ch + 1) * 128)
                                scch = normp.tile(
                                    [64, 128], BF16, tag="scch", bufs=4,
                                    name=f"scch{ch}")
                                nc.vector.tensor_mul(
                                    scch[:], av[1][0:64, cs], bc[1][:, cs])
                                nc.sync.dma_start(
                                    ot[pc][64:128, ocs], scch[:])
                                nc.vector.tensor_mul(
                                    ot[pc][0:64, ocs], av[0][0:64, cs],
                                    bc[0][:, cs])
                        else:
                            for h in (1, 0):
                                nc.vector.tensor_copy(orw[h][:], av[h][:])
                                nc.sync.dma_start(dd[h][:], orw[h][64:65, :])
                                nc.vector.reciprocal_approx_fast(
                                    rr[h][:], dd[h][:])
                                nc.gpsimd.partition_broadcast(
                                    bc[h][:], rr[h][:])
                                if h == 0:
                                    nc.vector.tensor_mul(
                                        ot[pc][0:64, js], orw[h][0:64, :],
                                        bc[h][:])
                                else:
                                    sc = normp.tile(
                                        [64, 512], BF16, tag="sc",
                                        bufs=4, name=f"sc{j}_{pc}")
                                    nc.vector.tensor_mul(
                                        sc[:], orw[h][0:64, :], bc[h][:])
                                    nc.sync.dma_start(
                                        ot[pc][64:128, js], sc[:])

                # complete the final q-chunk's output projection from the
                # staged partials (pair n-1's contribution + add + store).
                while filler:
                    filler.pop(0)[1]()
                jf = n_q - 1
                for st in range(4 * jf, 4 * jf + 4):
                    for cc in range(n_cc):
                        osb = out_proj(jf, st, cc, [n_pc - 1],
                                       add_to=osb_partial[(st, cc)])
                        dma_out(st, cc, osb)

    nc.compile()
    return nc


_NC_CACHE = {}


def _get_program():
    key = (S, D, HL)
    if key not in _NC_CACHE:
        _NC_CACHE[key] = build_program()
    return _NC_CACHE[key]


def _bf16(a):
    return np.ascontiguousarray(a.astype(ml_dtypes.bfloat16))


def _wtile(w):
    # [c*128, m] -> [128, c, m]: contraction chunk i lives at [:, i, :]
    c = w.shape[0] // 128
    return np.ascontiguousarray(
        w.reshape(c, 128, w.shape[1]).transpose(1, 0, 2).astype(
            ml_dtypes.bfloat16))


def _wtile_qk(w):
    # [c*128, p*128] -> [128, p, c, 128]: output chunk p is contiguous so the
    # ramp can load just the first head pair's weights.
    c = w.shape[0] // 128
    p = w.shape[1] // 128
    return np.ascontiguousarray(
        w.reshape(c, 128, p, 128).transpose(1, 2, 0, 3).astype(
            ml_dtypes.bfloat16))


def make_in_maps(X, Wq, Wk, Wv, Wo):
    in_maps = []
    for c in range(8):
        b, hg = c // 2, c % 2
        cs = slice(hg * DL, hg * DL + DL)
        in_maps.append({
            "XT": _bf16(X[b].T),
            "WQ": _wtile_qk(Wq[:, cs]),
            "WK": _wtile_qk(Wk[:, cs]),
            "WV": _wtile(Wv[:, cs]),
            "WO": _wtile(Wo[cs, :]),
        })
    return in_maps


def gather_out(results):
    out = np.empty((B, S, D), dtype=np.float32)
    for b in range(B):
        out[b] = results[2 * b]["OUT"] + results[2 * b + 1]["OUT"]
    return out


def kernel(X, Wq, Wk, Wv, Wo):
    X = np.asarray(X, dtype=np.float32)
    Wq = np.asarray(Wq, dtype=np.float32)
    Wk = np.asarray(Wk, dtype=np.float32)
    Wv = np.asarray(Wv, dtype=np.float32)
    Wo = np.asarray(Wo, dtype=np.float32)

    nc = _get_program()
    in_maps = make_in_maps(X, Wq, Wk, Wv, Wo)
    res = run_bass_kernel_spmd(nc, in_maps, list(range(8)), trace=False)
    return gather_out(res.results)


if __name__ == "__main__":
    rng = np.random.default_rng(0)
    scale = 1.0 / np.sqrt(D)
    inputs = {
        "X": rng.standard_normal((B, S, D), dtype=np.float32),
        "Wq": rng.standard_normal((D, D), dtype=np.float32) * scale,
        "Wk": rng.standard_normal((D, D), dtype=np.float32) * scale,
        "Wv": rng.standard_normal((D, D), dtype=np.float32) * scale,
        "Wo": rng.standard_normal((D, D), dtype=np.float32) * scale,
    }
    out = kernel(**inputs)
    print("kernel output shape:", out.shape)


# revision 62
# speedup vs baseline: 1.1019x; 1.0038x over previous
"""Trainium2 Bass kernel for multi-head causal self-attention.

Problem: X [4, 2048, 1024] fp32, Wq/Wk/Wv/Wo [1024, 1024], H=16 heads, HD=64.
reference: out = softmax_causal((X@Wq) (X@Wk)^T / 8) (X@Wv) merged @ Wo.

Sharding over 8 NeuronCores: core c handles batch b = c // 2 and head group
hg = c % 2 (8 heads each). Each core computes a partial [2048, 1024] output
(its heads' contribution through Wo's row shard); the host sums the two
partials per batch (the tensor-parallel all-reduce, done during unsharding).

Design notes (vs a phase-separated implementation; 432us -> 278us measured):
  * The ACT engine's exp throughput is the binding constraint of the
    attention inner loop, so all projection / output-projection work lives
    in a dependency-tagged filler queue drip-fed between attention steps:
    the PE never idles long enough for the HAM clock gate to re-throttle.
    Entries tagged with a unit are force-emitted before it (its score
    matmuls would FIFO-deadlock behind them otherwise).
  * The last two q-chunks' attention units are interleaved
    ((2,0),(2,1),(3,0),(2,2),(3,1),(2,3),(3,2),(3,3)) so the final,
    exp-heaviest chunk's ACT work starts ~25us early.
  * Scores for both heads of a pair go into one [128, 2, 512] fp32 PSUM
    tile (2 banks; score pairs run concurrently via tile_position row
    packing) so a single ACTIVATE handles exp for both heads.
  * Causal masking: one batched DVE add of a [128, 2, 128] -30000 triangle
    per diagonal k-block; fully-masked leading columns are simply never
    computed (scores, exp, and AV all operate on [rs:512]).
  * Normalization uses reciprocal_approx_fast (~5x faster than the
    microcoded reciprocal) + gpsimd partition_broadcast; AV-PSUM
    evacuation rides DVE. The very last unit normalizes straight from
    PSUM in 128-column chunks to shorten the kernel tail.
  * Host-side prep keeps every input DMA plain + contiguous: X arrives
    pre-transposed and is loaded in per-q-chunk column slices (phase 0
    unblocks after ~1.25 MB); Wq/Wk are pre-tiled per head-pair chunk.
"""

import sys

for _p in ("/opt/trn_rl_repo", "/root/.axon_site/_ro/trn_rl_repo"):
    if _p not in sys.path:
        sys.path.insert(0, _p)

import ml_dtypes
import numpy as np

import concourse.bass as bass
import concourse.mybir as mybir
import concourse.tile as tile
from concourse import bacc
from concourse.bass_utils import run_bass_kernel_spmd

F32 = mybir.dt.float32
BF16 = mybir.dt.bfloat16
EXPF = mybir.ActivationFunctionType.Exp

B, S, D, H = 4, 2048, 1024, 16
HD = D // H           # 64
HL = H // 2           # 8 heads per core
DL = HL * HD          # 512 local proj width
NEG = -30000.0        # causal mask additive value (exp underflows to 0)
VW = 65               # AV lhsT width: 64 V cols + ones col (denominator row)


def build_program(s=S, d=D, hl=HL):
    dl = hl * HD
    n_st = s // 128          # s-tiles (128 rows)
    n_dc = d // 128          # d-chunks (projection contraction)
    n_pc = dl // 128         # partition chunks (= head pairs)
    n_q = s // 512           # q-chunks
    n_cc = d // 512          # out column chunks

    nc = bacc.Bacc("TRN2", target_bir_lowering=False, debug=False)

    # X is fed pre-transposed and the weights pre-tiled by the host so every
    # input DMA is plain and contiguous (the XBAR transpose + scatter
    # rearrange DMAs dominated the ramp otherwise).
    XT = nc.dram_tensor("XT", [d, s], BF16, kind="ExternalInput")
    WQ = nc.dram_tensor("WQ", [128, n_pc, n_dc, 128], BF16,
                        kind="ExternalInput")
    WK = nc.dram_tensor("WK", [128, n_pc, n_dc, 128], BF16,
                        kind="ExternalInput")
    WV = nc.dram_tensor("WV", [128, n_dc, dl], BF16, kind="ExternalInput")
    WO = nc.dram_tensor("WO", [128, n_pc, d], BF16, kind="ExternalInput")
    OUT = nc.dram_tensor("OUT", [s, d], F32, kind="ExternalOutput")

    with tile.TileContext(nc) as tc:
        with tc.tile_pool(name="persist", bufs=1) as persist:
            # [128, 2, 128] additive causal mask for two stacked diagonal
            # blocks: 0 where q >= k else -30000.
            cmask = persist.tile([128, 2, 128], F32)
            nc.gpsimd.memset(cmask[:], 0.0)
            nc.gpsimd.affine_select(
                out=cmask[:], in_=cmask[:],
                compare_op=mybir.AluOpType.is_ge, fill=NEG,
                base=0, pattern=[[0, 2], [1, 128]], channel_multiplier=-1,
            )

            xt = [persist.tile([128, s], BF16, name=f"xt{i}") for i in range(n_dc)]
            wq = persist.tile([128, n_pc, n_dc, 128], BF16, name="wq")
            wk = persist.tile([128, n_pc, n_dc, 128], BF16, name="wk")
            wv = persist.tile([128, n_dc, dl], BF16, name="wv")
            wo = persist.tile([128, n_pc, d], BF16, name="wo")
            qt = [persist.tile([128, s], BF16, name=f"qt{i}") for i in range(n_pc)]
            kt = [persist.tile([128, s], BF16, name=f"kt{i}") for i in range(n_pc)]
            vt = [persist.tile([128, hl, VW], BF16, name=f"vt{i}")
                  for i in range(n_st)]
            ot = [persist.tile([128, s], BF16, name=f"ot{i}") for i in range(n_pc)]

            # All input loads ride the scalar HWDGE queue in dependency-
            # priority order; runtime DMAs (dd/sc/OUT) use the sync queue so
            # they never queue behind these. X^T comes in per-q-chunk column
            # slices: phase 0 only needs columns [0:512] (1 MB), so the first
            # attention unit unblocks ~20us earlier than with whole-tile
            # loads.
            nc.scalar.dma_start(wq[:, 0], WQ.ap()[:, 0])
            nc.scalar.dma_start(xt[0][:, 0:512], XT[0:128, 0:512])
            nc.scalar.dma_start(wk[:, 0], WK.ap()[:, 0])
            for dc in range(1, n_dc):
                nc.scalar.dma_start(
                    xt[dc][:, 0:512], XT[dc * 128:(dc + 1) * 128, 0:512])
            nc.scalar.dma_start(wv[:], WV.ap())
            for pc in range(1, n_pc):
                nc.scalar.dma_start(wq[:, pc], WQ.ap()[:, pc])
                nc.scalar.dma_start(wk[:, pc], WK.ap()[:, pc])
            for q in range(1, n_q):
                qs = slice(q * 512, (q + 1) * 512)
                for dc in range(n_dc):
                    nc.scalar.dma_start(
                        xt[dc][:, qs], XT[dc * 128:(dc + 1) * 128, qs])
                if q == 1:
                    nc.scalar.dma_start(wo[:], WO.ap())

            # exp table preload: emitting the first (dummy) activation here
            # makes walrus schedule the ~2.7us ACT_TABLE_LOAD during the
            # PE-heavy prologue instead of on the first attention chain.
            scr = persist.tile([128, 8], F32)
            nc.vector.memset(scr[:], 0.0)
            scr2 = persist.tile([128, 8], F32)
            nc.scalar.activation(scr2[:], scr[:], EXPF, scale=1.0)

            with (
                tc.tile_pool(name="pp", bufs=2, space="PSUM") as pp,
                tc.tile_pool(name="sp", bufs=2, space="PSUM") as sp,
                tc.tile_pool(name="avp", bufs=2, space="PSUM") as avp,
                tc.tile_pool(name="work", bufs=3) as work,
                tc.tile_pool(name="norm", bufs=4) as normp,
            ):
                def proj_v(st):
                    ps = pp.tile([128, dl], F32, tag="pp")
                    for dc in range(n_dc):
                        nc.tensor.matmul(
                            ps[:], xt[dc][:, st * 128:(st + 1) * 128],
                            wv[:, dc, :],
                            start=(dc == 0), stop=(dc == n_dc - 1))
                    nc.vector.memset(vt[st][:, :, 64:65], 1.0)
                    nc.vector.tensor_copy(
                        vt[st][:, :, 0:64],
                        ps[:].rearrange("p (h e) -> p h e", h=hl))

                def proj_qk(w, dst, pc, j1):
                    js1 = slice(j1 * 512, (j1 + 1) * 512)
                    ps = pp.tile([128, 512], F32, tag="pp")
                    for dc in range(n_dc):
                        nc.tensor.matmul(
                            ps[:], w[:, pc, dc, :], xt[dc][:, js1],
                            start=(dc == 0), stop=(dc == n_dc - 1))
                    nc.vector.tensor_copy(dst[pc][:, js1], ps[:])

                def out_proj(j, st, cc, pcs, add_to=None, staged=False):
                    """Partial output projection over head pairs `pcs`.
                    Returns the staged SBUF tile (caller DMAs or adds)."""
                    ps = pp.tile([128, 512], F32, tag="pp")
                    for n, pc in enumerate(pcs):
                        nc.tensor.matmul(
                            ps[:], ot[pc][:, st * 128:(st + 1) * 128],
                            wo[:, pc, cc * 512:(cc + 1) * 512],
                            start=(n == 0), stop=(n == len(pcs) - 1))
                    if add_to is None:
                        # the 8 last-chunk partials are all alive at once, so
                        # they get a dedicated 8-deep rotation (a 3-deep one
                        # FIFO-deadlocks DVE behind the final adds).
                        if staged:
                            osb = work.tile([128, 512], F32, tag="osbp",
                                            bufs=8, name=f"osbp{st}_{cc}")
                        else:
                            osb = work.tile([128, 512], F32, tag="osb",
                                            bufs=3, name=f"osb{st}_{cc}")
                        nc.vector.tensor_copy(osb[:], ps[:])
                        return osb
                    nc.vector.tensor_add(add_to[:], add_to[:], ps[:])
                    return add_to

                def dma_out(st, cc, osb):
                    nc.sync.dma_start(
                        OUT[st * 128:(st + 1) * 128, cc * 512:(cc + 1) * 512],
                        osb[:])

                # minimal prologue: just what attn(0, pc0) needs — Q/K for
                # pair 0 and the first four V tiles. The remaining j=0
                # projections ride the phase-0 filler.
                proj_qk(wq, qt, 0, 0)
                proj_qk(wk, kt, 0, 0)
                for st in range(4):
                    proj_v(st)

                # Unit sequence: the last two q-chunks' attention units are
                # interleaved so attn(3,*)'s exps (the ACT-bound stretch)
                # start ~25us earlier, overlapping attn(2,*)'s PE work.
                units = ([(0, pc) for pc in range(n_pc)]
                         + [(1, pc) for pc in range(n_pc)]
                         + [(2, 0), (2, 1), (3, 0), (2, 2), (3, 1), (2, 3),
                            (3, 2), (3, 3)])

                # filler: always-ready PE work drip-fed between attention
                # steps so the PE never starves while ACT exp gates the
                # dependency chain. Entries tagged with a unit are forced
                # out before that unit's emission (its scores would
                # FIFO-deadlock behind them otherwise).
                def qk_closure(w, dst, pc, j1):
                    return lambda: proj_qk(w, dst, pc, j1)

                def og_closure(jp, st, cc):
                    def og():
                        osb = out_proj(jp, st, cc, list(range(n_pc)))
                        dma_out(st, cc, osb)
                    return og

                appends = {u: [] for u in units}
                for pc in range(1, n_pc):
                    appends[(0, 0)].append(((0, pc), qk_closure(wq, qt, pc, 0)))
                    appends[(0, 0)].append(((0, pc), qk_closure(wk, kt, pc, 0)))
                for j1, host in ((1, (0, 1)), (2, (1, 0)), (3, (2, 0))):
                    for pc in range(n_pc):
                        appends[host].append(
                            ((j1, pc), qk_closure(wq, qt, pc, j1)))
                        appends[host].append(
                            ((j1, pc), qk_closure(wk, kt, pc, j1)))
                    for st in range(4 * j1, 4 * j1 + 4):
                        appends[host].append(((j1, 0), lambda st=st: proj_v(st)))
                for jp, host in ((0, (1, 0)), (1, (2, 0)), (2, (3, 2))):
                    for st in range(4 * jp, 4 * jp + 4):
                        for cc in range(n_cc):
                            appends[host].append((None, og_closure(jp, st, cc)))

                filler = []
                osb_partial = {}  # (st, cc) -> staged partial for the last j
                total_steps = sum(4 * u[0] + 4 for u in units)
                steps_done = 0
                step_ctr = 0
                for j, pc in units:
                    js = slice(j * 512, (j + 1) * 512)
                    n_i = 4 * j + 4
                    tail_unit = j == n_q - 1 and pc == n_pc - 1
                    filler += appends[(j, pc)]
                    rest = []
                    for tag, fn in filler:
                        if tag == (j, pc):
                            fn()
                        else:
                            rest.append((tag, fn))
                    filler = rest
                    if tail_unit:
                        # stage the partial output projection over pairs
                        # 0..n-2 while pair n-1 finishes its attention.
                        for st in range(4 * j, 4 * j + 4):
                            for cc in range(n_cc):
                                def frag(st=st, cc=cc, j=j):
                                    osb_partial[(st, cc)] = out_proj(
                                        j, st, cc, list(range(n_pc - 1)),
                                        staged=True)
                                filler.append((None, frag))
                    stride = max(1, (total_steps - steps_done)
                                 // max(1, len(filler)))
                    steps_done += n_i

                    if True:
                        av = [avp.tile([VW, 512], F32, tag="av",
                                       name=f"av{j}_{pc}_{h}") for h in (0, 1)]
                        ets = {}

                        def emit_av(i):
                            r = i - 4 * j
                            rs = max(r, 0) * 128
                            et = ets.pop(i)
                            for h in (0, 1):
                                nc.tensor.matmul(
                                    av[h][:, rs:512], vt[i][:, 2 * pc + h, :],
                                    et[:, h, rs:512],
                                    start=(i == 0), stop=(i == n_i - 1))

                        for i in range(n_i):
                            r = i - 4 * j
                            rs = max(r, 0) * 128
                            stp = sp.tile([128, 2, 512], F32, tag="sp")
                            for h in (0, 1):
                                nc.tensor.matmul(
                                    stp[:, h, rs:512],
                                    kt[pc][64 * h:64 * h + 64,
                                           i * 128:(i + 1) * 128],
                                    qt[pc][64 * h:64 * h + 64,
                                           j * 512 + rs:(j + 1) * 512],
                                    start=True, stop=True,
                                    tile_position=(64 * h, 0))
                            if r >= 0:
                                nc.vector.tensor_add(
                                    stp[:, :, rs:rs + 128],
                                    stp[:, :, rs:rs + 128], cmask[:])
                            et = work.tile([128, 2, 512], BF16, tag="et",
                                           bufs=4)
                            nc.scalar.activation(
                                et[:, :, rs:512], stp[:, :, rs:512], EXPF,
                                scale=0.125)
                            ets[i] = et
                            if i >= 2:
                                emit_av(i - 2)
                            step_ctr += 1
                            if filler and step_ctr % stride == 0:
                                filler.pop(0)[1]()
                        emit_av(n_i - 2)
                        emit_av(n_i - 1)
                        if tail_unit:
                            while filler:
                                filler.pop(0)[1]()

                        # normalization: denominators live in av row 64.
                        # h=1 first so its SBUF->SBUF partition-shift DMA
                        # overlaps h=0's DVE work. Only the very last unit
                        # evacuates the denominator row first (shortens the
                        # kernel tail); elsewhere a single copy releases the
                        # av PSUM bank as fast as possible.
                        orw, dd, rr, bc = {}, {}, {}, {}
                        for h in (1, 0):
                            orw[h] = normp.tile([VW, 512], F32, tag="orw",
                                                bufs=4, name=f"orw{j}_{pc}_{h}")
                            dd[h] = normp.tile([1, 512], F32, tag="dd", bufs=4,
                                               name=f"dd{j}_{pc}_{h}")
                            rr[h] = normp.tile([1, 512], F32, tag="rr", bufs=4,
                                               name=f"rr{j}_{pc}_{h}")
                            bc[h] = normp.tile([64, 512], F32, tag="bc", bufs=4,
                                               name=f"bc{j}_{pc}_{h}")
                        if tail_unit:
                            # shortest-latency tail: denominator rows out
                            # first, normalize straight from the AV PSUM (no
                            # one reuses those banks after the last unit),
                            # and chunk by 128 columns so the final output
                            # projection starts per-st instead of waiting
                            # for the full 512-wide partition-shift DMA.
                            for h in (1, 0):
                                nc.vector.tensor_copy(
                                    orw[h][64:65, :], av[h][64:65, :])
                                nc.sync.dma_start(dd[h][:], orw[h][64:65, :])
                            for h in (1, 0):
                                nc.vector.reciprocal_approx_fast(
                                    rr[h][:], dd[h][:])
                                nc.gpsimd.partition_broadcast(
                                    bc[h][:], rr[h][:])
                            for ch in range(4):
                                cs = slice(ch * 128, (ch + 1) * 128)
                                ocs = slice(j * 512 + ch * 128,
                                            j * 512 + (ch + 1) * 128)
                                scch = normp.tile(
                                    [64, 128], BF16, tag="scch", bufs=4,
                                    name=f"scch{ch}")
                                nc.vector.tensor_mul(
                                    scch[:], av[1][0:64, cs], bc[1][:, cs])
                                nc.sync.dma_start(
                                    ot[pc][64:128, ocs], scch[:])
                                nc.vector.tensor_mul(
                                    ot[pc][0:64, ocs], av[0][0:64, cs],
                                    bc[0][:, cs])
                        else:
                            for h in (1, 0):
                                nc.vector.tensor_copy(orw[h][:], av[h][:])
                                nc.sync.dma_start(dd[h][:], orw[h][64:65, :])
                                nc.vector.reciprocal_approx_fast(
                                    rr[h][:], dd[h][:])
                                nc.gpsimd.partition_broadcast(
                                    bc[h][:], rr[h][:])
                                if h == 0:
                                    nc.vector.tensor_mul(
                                        ot[pc][0:64, js], orw[h][0:64, :],
                                        bc[h][:])
                                else:
                                    sc = normp.tile(
                                        [64, 512], BF16, tag="sc",
                                        bufs=4, name=f"sc{j}_{pc}")
                                    nc.vector.tensor_mul(
                                        sc[:], orw[h][0:64, :], bc[h][:])
                                    nc.sync.dma_start(
                                        ot[pc][64:128, js], sc[:])

                # complete the final q-chunk's output projection from the
                # staged partials (pair n-1's contribution + add + store).
                while filler:
                    filler.pop(0)[1]()
                jf = n_q - 1
                for st in range(4 * jf, 4 * jf + 4):
                    for cc in range(n_cc):
                        osb = out_proj(jf, st, cc, [n_pc - 1],
                                       add_to=osb_partial[(st, cc)])
                        dma_out(st, cc, osb)

    nc.compile()
    return nc


_NC_CACHE = {}


def _get_program():
    key = (S, D, HL)
    if key not in _NC_CACHE:
        _NC_CACHE[key] = build_program()
    return _NC_CACHE[key]


def _bf16(a):
    return np.ascontiguousarray(a.astype(ml_dtypes.bfloat16))


def _wtile(w):
    # [c*128, m] -> [128, c, m]: contraction chunk i lives at [:, i, :]
    c = w.shape[0] // 128
    return np.ascontiguousarray(
        w.reshape(c, 128, w.shape[1]).transpose(1, 0, 2).astype(
            ml_dtypes.bfloat16))


def _wtile_qk(w):
    # [c*128, p*128] -> [128, p, c, 128]: output chunk p is contiguous so the
    # ramp can load just the first head pair's weights.
    c = w.shape[0] // 128
    p = w.shape[1] // 128
    return np.ascontiguousarray(
        w.reshape(c, 128, p, 128).transpose(1, 2, 0, 3).astype(
            ml_dtypes.bfloat16))


def make_in_maps(X, Wq, Wk, Wv, Wo):
    in_maps = []
    for c in range(8):
        b, hg = c // 2, c % 2
        cs = slice(hg * DL, hg * DL + DL)
        in_maps.append({
            "XT": _bf16(X[b].T),
            "WQ": _wtile_qk(Wq[:, cs]),
            "WK": _wtile_qk(Wk[:, cs]),
            "WV": _wtile(Wv[:, cs]),
            "WO": _wtile(Wo[cs, :]),
        })
    return in_maps


def gather_out(results):
    out = np.empty((B, S, D), dtype=np.float32)
    for b in range(B):
        out[b] = results[2 * b]["OUT"] + results[2 * b + 1]["OUT"]
    return out


def kernel(X, Wq, Wk, Wv, Wo):
    X = np.asarray(X, dtype=np.float32)
    Wq = np.asarray(Wq, dtype=np.float32)
    Wk = np.asarray(Wk, dtype=np.float32)
    Wv = np.asarray(Wv, dtype=np.float32)
    Wo = np.asarray(Wo, dtype=np.float32)

    nc = _get_program()
    in_maps = make_in_maps(X, Wq, Wk, Wv, Wo)
    res = run_bass_kernel_spmd(nc, in_maps, list(range(8)), trace=False)
    return gather_out(res.results)


if __name__ == "__main__":
    rng = np.random.default_rng(0)
    scale = 1.0 / np.sqrt(D)
    inputs = {
        "X": rng.standard_normal((B, S, D), dtype=np.float32),
        "Wq": rng.standard_normal((D, D), dtype=np.float32) * scale,
        "Wk": rng.standard_normal((D, D), dtype=np.float32) * scale,
        "Wv": rng.standard_normal((D, D), dtype=np.float32) * scale,
        "Wo": rng.standard_normal((D, D), dtype=np.float32) * scale,
    }
    out = kernel(**inputs)
    print("kernel output shape:", out.shape)
